# revision 1
# baseline (speedup 1.0000x reference)
"""TRN2 Bass kernel for BEiT-style attention (nn_Attention_27771258536423).

Strategy: data-parallel over batch across 8 NeuronCores (8 batches/core).
Per core:
  P0: build rel-pos bias [12][197,197] via one-hot matmuls over a staggered
      flipped bias table (no gathers, no negative-step DMAs).
  P1: qkv projection. q,k produced channel-major ([ch, tok], bf16, q pre-scaled
      and biased), v produced token-major ([tok, ch], bf16, biased). f32r matmuls.
  P2: per (batch, head): scores S = qT.T @ kT -> +bias -> exp (accum denominator)
      -> renormalized exp via bias=-ln(denom) -> PE-transpose E -> E.T
      -> attn_out.T = v.T-chunks @ E.T (channel-major f32). bf16 matmuls.
  P3: y = attn_out.T.T @ proj_w.T + proj_b, token-major f32 out. f32r matmuls.
"""
import sys

sys.path.insert(0, '/opt/trn_rl_repo')

import numpy as np
import ml_dtypes

import concourse.bass as bass
import concourse.mybir as mybir
import concourse.tile as tile
from concourse import bacc
from concourse.masks import make_identity
dt = mybir.dt
BF16 = ml_dtypes.bfloat16

DIM = 768
NH = 12
HD = 64
N_TOK = 197
SCALE = HD ** (-0.5)
TW = 736
OH_CHUNKS = [(0, 128), (128, 128), (256, 109)]   # (jbase, size) over j in [0,365)
N1C = [(0, 128), (128, 69)]                       # token partition chunks

_cache = {}


def _ap(t, offset, ap):
    return bass.AP(tensor=t.tensor if hasattr(t, 'tensor') else t,
                   offset=offset, ap=ap)


def build_program(nb):
    """nb = batches per core. Returns (nc, out_name)."""
    ntok = nb * N_TOK
    nfull, nrem = divmod(ntok, 128)
    tok_chunks = [(128 * i, 128) for i in range(nfull)]
    if nrem:
        tok_chunks.append((128 * nfull, nrem))
    # qkv N-chunks over tokens: pairs of rows (394) when possible
    qkv_nc = []
    o = 0
    while o < ntok:
        w = min(394, ntok - o)
        qkv_nc.append((o, w))
        o += w

    nc = bacc.Bacc(None)

    xT_d = nc.dram_tensor("xT", [DIM, ntok], dt.float32, kind="ExternalInput")
    wqkvT_d = nc.dram_tensor("wqkvT", [DIM, 3 * DIM], dt.float32, kind="ExternalInput")
    wprojT_d = nc.dram_tensor("wprojT", [DIM, DIM], dt.float32, kind="ExternalInput")
    qb2_d = nc.dram_tensor("qb2", [128, 6], dt.float32, kind="ExternalInput")
    vb_d = nc.dram_tensor("vb", [DIM], dt.float32, kind="ExternalInput")
    pb_d = nc.dram_tensor("pb", [DIM], dt.float32, kind="ExternalInput")
    tabF_d = nc.dram_tensor("tabF", [NH, TW], dt.bfloat16, kind="ExternalInput")
    onehot_d = nc.dram_tensor("onehot", [365, N_TOK], dt.bfloat16, kind="ExternalInput")
    clsrow_d = nc.dram_tensor("clsrow", [NH, N_TOK], dt.float32, kind="ExternalInput")
    clscol_d = nc.dram_tensor("clscol", [NH], dt.float32, kind="ExternalInput")
    y_d = nc.dram_tensor("y", [ntok, DIM], dt.float32, kind="ExternalOutput")

    f32r = dt.float32r
    Exp = mybir.ActivationFunctionType.Exp
    Ln = mybir.ActivationFunctionType.Ln
    Ident = mybir.ActivationFunctionType.Identity

    with tile.TileContext(nc) as tc:
        import contextlib
        with contextlib.ExitStack() as stk:
            consts = stk.enter_context(tc.tile_pool(name="consts", bufs=1))
            biasp = stk.enter_context(tc.tile_pool(name="biasp", bufs=1))
            qkp = stk.enter_context(tc.tile_pool(name="qkp", bufs=1))
            vp = stk.enter_context(tc.tile_pool(name="vp", bufs=1))

            # ---------- constants ----------
            oh_sb = consts.tile([128, 3 * N_TOK], dt.bfloat16, name="oh", tag="oh")
            for c, (jb, sz) in enumerate(OH_CHUNKS):
                nc.sync.dma_start(out=oh_sb[0:sz, c * N_TOK:(c + 1) * N_TOK],
                                  in_=onehot_d[jb:jb + sz, :])
            qb2_sb = consts.tile([128, 6], dt.float32, name="qb2", tag="qb2")
            nc.sync.dma_start(out=qb2_sb[:, :], in_=qb2_d[:, :])
            vb_rep = consts.tile([128, DIM], dt.float32, name="vbrep", tag="vbrep")
            nc.sync.dma_start(out=vb_rep[:, :],
                              in_=_ap(vb_d, 0, [[0, 128], [1, DIM]]))
            pb_rep = consts.tile([128, DIM], dt.float32, name="pbrep", tag="pbrep")
            nc.sync.dma_start(out=pb_rep[:, :],
                              in_=_ap(pb_d, 0, [[0, 128], [1, DIM]]))
            clsrowT = consts.tile([1, NH * N_TOK], dt.float32, name="clsrowT", tag="clsrowT")
            nc.sync.dma_start(out=clsrowT[0:1, :],
                              in_=_ap(clsrow_d, 0, [[NH * N_TOK, 1], [1, NH * N_TOK]]))
            clscol_sb = consts.tile([128, NH], dt.float32, name="clscol", tag="clscol")
            nc.sync.dma_start(out=clscol_sb[:, :],
                              in_=_ap(clscol_d, 0, [[0, 128], [1, NH]]))
            ident = consts.tile([128, 128], dt.bfloat16, name="ident", tag="ident")
            make_identity(nc, ident)

            # ---------- P0: bias build ----------
            bias_sb = {}   # (h, ci) -> tile [n1c, 197] f32
            with (tc.tile_pool(name="stagp", bufs=2) as stagp,
                  tc.tile_pool(name="biasps", bufs=2, space="PSUM") as biasps):
                for h in range(NH):
                    stags = []
                    for c, (jb, sz) in enumerate(OH_CHUNKS):
                        A = 364 - jb - sz + 1
                        st = stagp.tile([128, 365], dt.bfloat16, name=f"stag{c}", tag=f"stag{c}")
                        nc.sync.dma_start(out=st[0:sz, :],
                                          in_=_ap(tabF_d, h * TW + A,
                                                  [[1, sz], [1, 365]]))
                        stags.append(st)
                    for ci, (n1o, n1c) in enumerate(N1C):
                        bt = biasp.tile([n1c, N_TOK], dt.bfloat16, name=f"b{h}_{ci}", tag=f"b{h}_{ci}")
                        bias_sb[(h, ci)] = bt
                        bp = biasps.tile([n1c, 196], dt.float32, name="bps", tag="bps")
                        for c, (jb, sz) in enumerate(OH_CHUNKS):
                            st = stags[c]
                            rhs = _ap(st, st.offset,
                                      [[st.ap[0][0], sz], [27, 14], [1, 14]])
                            nc.tensor.matmul(
                                bp[:, :],
                                oh_sb[0:sz, c * N_TOK + n1o:c * N_TOK + n1o + n1c],
                                rhs, start=(c == 0), stop=(c == 2))
                        # col n2=0 first, then psum block, then (ci==0) row 0
                        nc.vector.tensor_copy(bt[0:n1c, 0:1],
                                              clscol_sb[0:n1c, h:h + 1])
                        nc.vector.tensor_copy(bt[0:n1c, 1:N_TOK], bp[:, :])
                        if ci == 0:
                            nc.vector.tensor_copy(
                                bt[0:1, 0:N_TOK],
                                clsrowT[0:1, h * N_TOK:(h + 1) * N_TOK])

            # ---------- P1: qkv ----------
            qk_sb = []   # 12 tiles [128, ntok] bf16: ch-major; 0-5 q, 6-11 k
            for t in range(12):
                qk_sb.append(qkp.tile([128, ntok], dt.bfloat16, name=f"qk{t}", tag=f"qk{t}"))
            v_sb = []    # per batch: per tok-chunk-of-batch tiles [<=128, 768] bf16
            for b in range(nb):
                v_sb.append([vp.tile([128, DIM], dt.bfloat16, name=f"v{b}_0", tag=f"v{b}_0"),
                             vp.tile([69, DIM], dt.bfloat16, name=f"v{b}_1", tag=f"v{b}_1")])

            with (tc.tile_pool(name="xp", bufs=1) as xp,
                  tc.tile_pool(name="wqp", bufs=1) as wqp,
                  tc.tile_pool(name="mmps", bufs=4, space="PSUM") as mmps):
                xT = []
                xTb = []
                for k in range(6):
                    xt = xp.tile([128, ntok], dt.float32r, name=f"x{k}", tag=f"x{k}")
                    for (no, nw) in qkv_nc:
                        nc.sync.dma_start(
                            out=xt[:, no:no + nw],
                            in_=xT_d[128 * k:128 * (k + 1), no:no + nw]
                            .bitcast(dt.float32r))
                    xT.append(xt)
                    # bf16 copy of x for the q/k matmuls (gpsimd DMA casts)
                    xtb = xp.tile([128, ntok], dt.bfloat16, name=f"xb{k}", tag=f"xb{k}")
                    for (no, nw) in qkv_nc:
                        nc.gpsimd.dma_start(
                            out=xtb[:, no:no + nw],
                            in_=xT_d[128 * k:128 * (k + 1), no:no + nw])
                    xTb.append(xtb)
                wq = []      # v columns only, f32r [128, 768]
                wqb = []     # qk columns, bf16 [128, 1536]
                for k in range(6):
                    wt = wqp.tile([128, DIM], dt.float32r, name=f"wq{k}", tag=f"wq{k}")
                    nc.sync.dma_start(
                        out=wt[:, :],
                        in_=wqkvT_d[128 * k:128 * (k + 1), 1536:2304]
                        .bitcast(dt.float32r))
                    wq.append(wt)
                    wtb = wqp.tile([128, 2 * DIM], dt.bfloat16, name=f"wqb{k}", tag=f"wqb{k}")
                    for c0 in (0, 768):
                        nc.gpsimd.dma_start(
                            out=wtb[:, c0:c0 + 768],
                            in_=wqkvT_d[128 * k:128 * (k + 1), c0:c0 + 768])
                    wqb.append(wtb)

                # q,k channel-major
                for m in range(12):
                    for (no, nw) in qkv_nc:
                        ps = mmps.tile([128, 394], dt.float32, name="qkps", tag="qkps")
                        for k in range(6):
                            nc.tensor.matmul(
                                ps[:, 0:nw],
                                wqb[k][:, 128 * m:128 * (m + 1)],
                                xTb[k][:, no:no + nw],
                                start=(k == 0), stop=(k == 5))
                        if m < 6:   # q: (x+qb)*scale on DVE
                            nc.vector.tensor_scalar(
                                out=qk_sb[m][:, no:no + nw], in0=ps[:, 0:nw],
                                scalar1=qb2_sb[:, m:m + 1], scalar2=float(SCALE),
                                op0=mybir.AluOpType.add,
                                op1=mybir.AluOpType.mult)
                        else:       # k: plain cast copy on DVE
                            nc.vector.tensor_copy(qk_sb[m][:, no:no + nw],
                                                  ps[:, 0:nw])

                # v token-major per batch
                for b in range(nb):
                    for ci, (to, tw_) in enumerate(((0, 128), (128, 69))):
                        for half in range(2):
                            ps = mmps.tile([128, 394], dt.float32, name="qkps", tag="qkps")
                            for k in range(6):
                                nc.tensor.matmul(
                                    ps[0:tw_, 0:384],
                                    xT[k][:, N_TOK * b + to:N_TOK * b + to + tw_],
                                    wq[k][:, 384 * half:384 * (half + 1)],
                                    start=(k == 0), stop=(k == 5))
                            nc.vector.tensor_tensor(
                                out=v_sb[b][ci][0:tw_, 384 * half:384 * (half + 1)],
                                in0=ps[0:tw_, 0:384],
                                in1=vb_rep[0:tw_, 384 * half:384 * (half + 1)],
                                op=mybir.AluOpType.add)

            # ---------- P2: attention ----------
            aop = stk.enter_context(tc.tile_pool(name="aout", bufs=1))
            attn_outT = []
            for t in range(6):
                attn_outT.append(aop.tile([128, ntok], dt.float32r, name=f"ao{t}", tag=f"ao{t}"))

            with (tc.tile_pool(name="ssb", bufs=3) as ssbp,
                  tc.tile_pool(name="esb", bufs=4) as esbp,
                  tc.tile_pool(name="etp", bufs=4) as etp,
                  tc.tile_pool(name="dnp", bufs=8) as dnp,
                  tc.tile_pool(name="sps", bufs=3, space="PSUM") as sps,
                  tc.tile_pool(name="tps", bufs=3, space="PSUM") as tps,
                  tc.tile_pool(name="avps", bufs=2, space="PSUM") as avps):
                # S-chunk column offsets inside one packed psum bank tile
                SOFF = [0, 256]            # f32 [128,512]: c0 0:197, c1 256:453
                TOFF = [0, 512]            # bf16 [128,1024]: cj0 0:197, cj1 512:709
                for b in range(nb):
                    for hp in range(NH // 2):
                        qt = qk_sb[hp]
                        kt = qk_sb[6 + hp]
                        # head pair (2*hp, 2*hp+1); interleave the two heads so
                        # their K=64 / M=64 matmuls sit adjacent (row/col-group
                        # concurrency in the PE array)
                        sp2 = [sps.tile([128, 512], dt.float32, name="sps",
                                        tag="sps") for _ in range(2)]
                        # scores q.k (PE); bias added on DVE into sbuf
                        ssb2 = [[], []]
                        for ci, (n1o, n1c) in enumerate(N1C):
                            for hi in range(2):
                                po = hi * 64
                                nc.tensor.matmul(
                                    sp2[hi][0:n1c, SOFF[ci]:SOFF[ci] + N_TOK],
                                    qt[po:po + 64,
                                       N_TOK * b + n1o:N_TOK * b + n1o + n1c],
                                    kt[po:po + 64, N_TOK * b:N_TOK * (b + 1)],
                                    start=True, stop=True)
                        for ci, (n1o, n1c) in enumerate(N1C):
                            for hi in range(2):
                                ss = ssbp.tile([n1c, N_TOK], dt.float32,
                                               name=f"ss{ci}", tag=f"ss{ci}{hi}")
                                nc.vector.tensor_tensor(
                                    out=ss[:, :],
                                    in0=sp2[hi][0:n1c, SOFF[ci]:SOFF[ci] + N_TOK],
                                    in1=bias_sb[(2 * hp + hi, ci)][:, :],
                                    op=mybir.AluOpType.add)
                                ssb2[hi].append(ss)
                        # softmax (free dim): exp + DVE renormalize
                        esb2 = [[], []]
                        for ci, (n1o, n1c) in enumerate(N1C):
                            for hi in range(2):
                                e = esbp.tile([n1c, N_TOK], dt.bfloat16,
                                              name=f"e{ci}", tag=f"e{ci}{hi}")
                                dsum = dnp.tile([n1c, 1], dt.float32,
                                                name=f"d{ci}", tag=f"d{ci}{hi}")
                                nc.scalar.activation(
                                    out=e[:, :], in_=ssb2[hi][ci][:, :],
                                    func=Exp, accum_out=dsum[:, :])
                                rec = dnp.tile([n1c, 1], dt.float32,
                                               name=f"r{ci}", tag=f"r{ci}{hi}")
                                nc.vector.reciprocal(rec[:, :], dsum[:, :])
                                en = esbp.tile([n1c, N_TOK], dt.bfloat16,
                                               name=f"en{ci}", tag=f"en{ci}{hi}")
                                nc.vector.tensor_scalar(
                                    out=en[:, :], in0=e[:, :],
                                    scalar1=rec[:, :], scalar2=None,
                                    op0=mybir.AluOpType.mult)
                                esb2[hi].append(en)
                        # transpose E -> E_T; both heads' blocks interleaved
                        tp2 = [tps.tile([128, 1024], dt.bfloat16, name="tps",
                                        tag="tps") for _ in range(2)]
                        et2 = [[etp.tile([128, N_TOK], dt.bfloat16,
                                         name="et0", tag=f"et0{hi}"),
                                etp.tile([69, N_TOK], dt.bfloat16,
                                         name="et1", tag=f"et1{hi}")]
                               for hi in range(2)]
                        for cj, (n2o, n2c) in enumerate(N1C):
                            for ci, (n1o, n1c) in enumerate(N1C):
                                for hi in range(2):
                                    nc.tensor.transpose(
                                        tp2[hi][0:n2c,
                                                TOFF[cj] + n1o:TOFF[cj] + n1o + n1c],
                                        esb2[hi][ci][:, n2o:n2o + n2c],
                                        ident[0:n1c, 0:n1c])
                            for hi in range(2):
                                nc.vector.tensor_copy(
                                    et2[hi][cj][:, :],
                                    tp2[hi][0:n2c, TOFF[cj]:TOFF[cj] + N_TOK])
                        # av into the pair psum halves (col-group concurrency)
                        ap_ = avps.tile([128, 512], dt.float32, name="avps",
                                        tag="avps")
                        for hi in range(2):
                            h = 2 * hp + hi
                            po = hi * 64
                            for cj, (n2o, n2c) in enumerate(N1C):
                                nc.tensor.matmul(
                                    ap_[po:po + 64, 0:N_TOK],
                                    v_sb[b][cj][:, HD * h:HD * (h + 1)],
                                    et2[hi][cj][:, :],
                                    start=(cj == 0), stop=(cj == 1))
                        nc.vector.tensor_copy(
                            attn_outT[hp][:, N_TOK * b:N_TOK * (b + 1)],
                            ap_[:, 0:N_TOK])

            # ---------- P3: proj ----------
            with (tc.tile_pool(name="wpp", bufs=1) as wpp,
                  tc.tile_pool(name="ysb", bufs=3) as ysbp,
                  tc.tile_pool(name="pps", bufs=4, space="PSUM") as pps):
                wp = []
                for k in range(6):
                    wt = wpp.tile([128, DIM], dt.float32r, name=f"wp{k}", tag=f"wp{k}")
                    nc.sync.dma_start(out=wt[:, :],
                                      in_=wprojT_d[128 * k:128 * (k + 1), :].bitcast(dt.float32r))
                    wp.append(wt)
                for (to, tw_) in tok_chunks:
                    ys = ysbp.tile([128, DIM], dt.float32, name="ys", tag="ys")
                    for half in range(2):
                        ps = pps.tile([128, 384], dt.float32, name="pps", tag="pps")
                        for k in range(6):
                            nc.tensor.matmul(
                                ps[0:tw_, :],
                                attn_outT[k][:, to:to + tw_],
                                wp[k][:, 384 * half:384 * (half + 1)],
                                start=(k == 0), stop=(k == 5))
                        nc.vector.tensor_tensor(
                            out=ys[0:tw_, 384 * half:384 * (half + 1)],
                            in0=ps[0:tw_, :],
                            in1=pb_rep[0:tw_, 384 * half:384 * (half + 1)],
                            op=mybir.AluOpType.add)
                    nc.sync.dma_start(out=y_d[to:to + tw_, :], in_=ys[0:tw_, :])

    nc.compile()
    return nc


def _marshal(x, qkv_w, q_bias, v_bias, rpb_table, proj_w, proj_b, rel_index):
    B = x.shape[0]
    ncore = 8
    bpc = B // ncore
    x2 = np.ascontiguousarray(x.reshape(B, N_TOK, DIM))

    wqkvT = np.ascontiguousarray(qkv_w.T.astype(np.float32))
    wprojT = np.ascontiguousarray(proj_w.T.astype(np.float32))
    qb2 = np.ascontiguousarray(q_bias.astype(np.float32).reshape(6, 128).T)
    tabF = np.zeros((NH, TW), dtype=BF16)
    tabF[:, 0:729] = rpb_table[728::-1, :].T.astype(BF16)
    tabF[:, 729:732] = rpb_table[729:732, :].T.astype(BF16)
    clsrow = np.zeros((NH, N_TOK), dtype=np.float32)
    clsrow[:, 0] = rpb_table[731, :]
    clsrow[:, 1:] = rpb_table[729, :][:, None]
    clscol = np.ascontiguousarray(rpb_table[730, :].astype(np.float32))
    onehot = np.zeros((365, N_TOK), dtype=BF16)
    for y1 in range(14):
        for x1 in range(14):
            c1 = 27 * y1 + x1
            n1 = 1 + 14 * y1 + x1
            for (jb, sz) in OH_CHUNKS:
                if jb <= c1 < jb + sz:
                    onehot[jb + (jb + sz - 1 - c1), n1] = 1

    shared = {"wqkvT": wqkvT, "wprojT": wprojT, "qb2": qb2,
              "vb": np.ascontiguousarray(v_bias.astype(np.float32)),
              "pb": np.ascontiguousarray(proj_b.astype(np.float32)),
              "tabF": tabF, "onehot": onehot,
              "clsrow": clsrow, "clscol": clscol}
    in_maps = []
    for c in range(ncore):
        xT = np.ascontiguousarray(
            x2[c * bpc:(c + 1) * bpc].reshape(bpc * N_TOK, DIM).T)
        m = dict(shared)
        m["xT"] = xT
        in_maps.append(m)
    return in_maps, bpc


last_exec_time_ns = None
last_results = None


def _install_ntff_hook():
    """Provide antenv.axon_hooks + register the ctypes NTFF hook (the agent
    image's antenv lacks axon_hooks, so trn_boot degraded silently)."""
    import types
    import contextlib
    import ctypes

    try:
        from antenv.axon_hooks import get_axon_ntff_profile_hook
        if get_axon_ntff_profile_hook() is not None:
            return
    except ImportError:
        import antenv
        mod = types.ModuleType("antenv.axon_hooks")
        mod._hook = None

        def set_axon_ntff_profile_hook(h):
            mod._hook = h

        def get_axon_ntff_profile_hook():
            return mod._hook

        mod.set_axon_ntff_profile_hook = set_axon_ntff_profile_hook
        mod.get_axon_ntff_profile_hook = get_axon_ntff_profile_hook
        sys.modules["antenv.axon_hooks"] = mod
        antenv.axon_hooks = mod

    so_path = "/opt/axon/libaxon_pjrt.so"
    lib = ctypes.CDLL(so_path)
    if not hasattr(lib, "axon_start_nrt_profile"):
        return
    lib.axon_start_nrt_profile.argtypes = [ctypes.POINTER(ctypes.c_int64),
                                           ctypes.c_size_t]
    lib.axon_start_nrt_profile.restype = ctypes.c_int64
    lib.axon_stop_nrt_profile.argtypes = [ctypes.c_char_p]
    lib.axon_stop_nrt_profile.restype = ctypes.c_int64

    @contextlib.contextmanager
    def _hook(output_dir, device_ids):
        import jax
        jax.devices()
        if device_ids:
            ids = (ctypes.c_int64 * len(device_ids))(*device_ids)
            rc = lib.axon_start_nrt_profile(ids, len(device_ids))
        else:
            rc = lib.axon_start_nrt_profile(None, 0)
        if rc != 0:
            raise RuntimeError(f"axon_start_nrt_profile rc={rc}")
        try:
            yield
        finally:
            n = lib.axon_stop_nrt_profile(str(output_dir).encode())
            print(f"ntff profile: {n} file(s) -> {output_dir}", file=sys.stderr)

    from antenv.axon_hooks import set_axon_ntff_profile_hook
    set_axon_ntff_profile_hook(_hook)


def kernel(x, qkv_w, q_bias, v_bias, rpb_table, proj_w, proj_b, rel_index):
    global last_exec_time_ns
    import os
    if os.environ.get("KERNEL_TRACE"):
        _install_ntff_hook()
    from concourse.bass_utils import run_bass_kernel_spmd

    x = np.asarray(x, dtype=np.float32)
    qkv_w = np.asarray(qkv_w, dtype=np.float32)
    q_bias = np.asarray(q_bias, dtype=np.float32)
    v_bias = np.asarray(v_bias, dtype=np.float32)
    rpb_table = np.asarray(rpb_table, dtype=np.float32)
    proj_w = np.asarray(proj_w, dtype=np.float32)
    proj_b = np.asarray(proj_b, dtype=np.float32)

    B = x.shape[0]
    bpc = B // 8
    if 'nc' not in _cache:
        _cache['nc'] = build_program(bpc)
    nc = _cache['nc']

    in_maps, bpc = _marshal(x, qkv_w, q_bias, v_bias, rpb_table,
                            proj_w, proj_b, rel_index)
    import os
    res = run_bass_kernel_spmd(nc, in_maps, core_ids=list(range(8)),
                               trace=bool(os.environ.get("KERNEL_TRACE")))
    last_exec_time_ns = res.exec_time_ns
    global last_results
    last_results = res
    ys = [res.results[c]["y"].reshape(bpc, N_TOK, DIM) for c in range(8)]
    return np.concatenate(ys, axis=0).astype(np.float32)



# revision 19
# speedup vs baseline: 1.1094x; 1.1094x over previous
"""TRN2 Bass kernel for BEiT-style attention (nn_Attention_27771258536423).

Strategy: data-parallel over batch across 8 NeuronCores (8 batches/core).
Per core (all matmuls bf16, psum f32):
  - rel-pos bias precomputed on host, shipped TRANSPOSED and pair-packed.
  - qkv: q,k channel-major [ch, tok] bf16 (q pre-scaled+biased via folded
    weights); v token-major [tok, 65*12] bf16 with a ones column per head
    (denominator rides along in the AV matmul).
  - attention per (batch, head-pair): S^T = k^T q directly (no transposes
    anywhere: exp(S^T + bias^T) IS E^T, the AV moving operand).
    AV: out[65, 197] = V_aug^T E^T -> rows 0:64 = attn_out^T (channel-major),
    row 64 = softmax denominator. reciprocal (DVE) -> partition_broadcast
    (gpsimd) -> multiply (DVE) into attn_outT bf16.
  - proj per batch, interleaved into the attention stream as PE filler
    together with next-chunk qkv matmuls (covers softmax latency).
"""
import sys

sys.path.insert(0, '/opt/trn_rl_repo')

import numpy as np
import ml_dtypes

import concourse.bass as bass
import concourse.mybir as mybir
import concourse.tile as tile
from concourse import bacc

dt = mybir.dt
BF16 = ml_dtypes.bfloat16

DIM = 768
NH = 12
HD = 64
N_TOK = 197
SCALE = HD ** (-0.5)
CHUNK = 2 * N_TOK          # 394 tokens = 2 batches per qkv chunk
N1C = [(0, 128), (128, 69)]  # token chunks within one batch (n2 chunks)

_cache = {}


def _ap(t, offset, ap):
    return bass.AP(tensor=t.tensor if hasattr(t, 'tensor') else t,
                   offset=offset, ap=ap)


def build_program(nb, debug=False, feats="scores,bias,exp,av,rt,proj,fill"):
    F = set(feats.split(",")) if feats else set()
    """nb = batches per core (8). Returns compiled Bacc."""
    assert nb % 2 == 0
    ntok = nb * N_TOK
    nchunks = nb // 2

    nc = bacc.Bacc(None)
    if debug:
        dbg_qk_d = nc.dram_tensor("dbg_qk", [12 * 128, ntok], dt.bfloat16,
                                  kind="ExternalOutput")
        dbg_v_d = nc.dram_tensor("dbg_v", [2 * 128, 65 * NH], dt.bfloat16,
                                 kind="ExternalOutput")
        dbg_ss0_d = nc.dram_tensor("dbg_ss0", [128, 394], dt.float32,
                                   kind="ExternalOutput")
        dbg_ss1_d = nc.dram_tensor("dbg_ss1", [69, 394], dt.float32,
                                   kind="ExternalOutput")
        dbg_et0_d = nc.dram_tensor("dbg_et0", [128, 394], dt.bfloat16,
                                   kind="ExternalOutput")
        dbg_et1_d = nc.dram_tensor("dbg_et1", [69, 394], dt.bfloat16,
                                   kind="ExternalOutput")
        dbg_rc_d = nc.dram_tensor("dbg_rc", [1, 394], dt.float32,
                                  kind="ExternalOutput")
        dbg_rb_d = nc.dram_tensor("dbg_rb", [64, 394], dt.float32,
                                  kind="ExternalOutput")
        dbg_ao_d = nc.dram_tensor("dbg_ao", [6 * 128, ntok], dt.bfloat16,
                                  kind="ExternalOutput")

    xTb_d = nc.dram_tensor("xTb", [DIM, ntok], dt.bfloat16, kind="ExternalInput")
    wqkT_d = nc.dram_tensor("wqkT", [DIM, 2 * DIM], dt.bfloat16, kind="ExternalInput")
    wvT_d = nc.dram_tensor("wvT", [DIM, DIM], dt.bfloat16, kind="ExternalInput")
    wpT_d = nc.dram_tensor("wpT", [DIM, DIM], dt.bfloat16, kind="ExternalInput")
    qb2_d = nc.dram_tensor("qb2", [128, 6], dt.float32, kind="ExternalInput")
    vb_d = nc.dram_tensor("vb", [DIM], dt.float32, kind="ExternalInput")
    pb_d = nc.dram_tensor("pb", [DIM], dt.float32, kind="ExternalInput")
    bT0_d = nc.dram_tensor("bT0", [6 * 128, 394], dt.bfloat16, kind="ExternalInput")
    bT1_d = nc.dram_tensor("bT1", [6 * 69, 394], dt.bfloat16, kind="ExternalInput")
    y_d = nc.dram_tensor("y", [ntok, DIM], dt.float32, kind="ExternalOutput")

    Exp = mybir.ActivationFunctionType.Exp

    with tile.TileContext(nc) as tc:
        import contextlib
        with contextlib.ExitStack() as stk:
            consts = stk.enter_context(tc.tile_pool(name="consts", bufs=1))
            wpool = stk.enter_context(tc.tile_pool(name="wpool", bufs=1))
            xp = stk.enter_context(tc.tile_pool(name="xp", bufs=1))
            qkp = stk.enter_context(tc.tile_pool(name="qkp", bufs=1))
            vp = stk.enter_context(tc.tile_pool(name="vp", bufs=1))
            aop = stk.enter_context(tc.tile_pool(name="aop", bufs=1))
            ss0p = stk.enter_context(tc.tile_pool(name="ss0p", bufs=3))
            ss1p = stk.enter_context(tc.tile_pool(name="ss1p", bufs=3))
            et0p = stk.enter_context(tc.tile_pool(name="et0p", bufs=3))
            et1p = stk.enter_context(tc.tile_pool(name="et1p", bufs=3))
            rcp = stk.enter_context(tc.tile_pool(name="rcp", bufs=3))
            rbp = stk.enter_context(tc.tile_pool(name="rbp", bufs=3))
            ysp = stk.enter_context(tc.tile_pool(name="ysp", bufs=3))
            mmps = stk.enter_context(tc.tile_pool(name="mmps", bufs=5, space="PSUM"))
            avps = stk.enter_context(tc.tile_pool(name="avps", bufs=3, space="PSUM"))
            dramp = stk.enter_context(tc.tile_pool(name="dramp", bufs=4, space="DRAM"))

            # ---------- constant / weight DMAs ----------
            # x chunk 0 + q-weight tiles first (first matmuls need them)
            xb = [xp.tile([128, ntok], dt.bfloat16, name=f"xb{k}", tag=f"xb{k}")
                  for k in range(6)]
            for k in range(6):
                nc.sync.dma_start(out=xb[k][:, 0:CHUNK],
                                  in_=xTb_d[128 * k:128 * (k + 1), 0:CHUNK])
            wqk = [wpool.tile([128, 2 * DIM], dt.bfloat16, name=f"wqk{k}",
                              tag=f"wqk{k}") for k in range(6)]
            for k in range(6):   # q columns first
                nc.sync.dma_start(out=wqk[k][:, 0:DIM],
                                  in_=wqkT_d[128 * k:128 * (k + 1), 0:DIM])
            qb2_sb = consts.tile([128, 6], dt.float32, name="qb2", tag="qb2")
            nc.sync.dma_start(out=qb2_sb[:, :], in_=qb2_d[:, :])
            for k in range(6):   # k columns
                nc.sync.dma_start(out=wqk[k][:, DIM:2 * DIM],
                                  in_=wqkT_d[128 * k:128 * (k + 1), DIM:2 * DIM])
            wv = [wpool.tile([128, DIM], dt.bfloat16, name=f"wv{k}", tag=f"wv{k}")
                  for k in range(6)]
            for k in range(6):
                nc.sync.dma_start(out=wv[k][:, :],
                                  in_=wvT_d[128 * k:128 * (k + 1), :])
            vb_rep = consts.tile([128, DIM], dt.float32, name="vbrep", tag="vbrep")
            nc.sync.dma_start(out=vb_rep[:, :],
                              in_=_ap(vb_d, 0, [[0, 128], [1, DIM]]))
            # bias tiles (needed at attention time)
            bT0_sb = []
            bT1_sb = []
            for hp in range(6):
                b0 = consts.tile([128, 394], dt.bfloat16, name=f"bT0_{hp}",
                                 tag=f"bT0_{hp}")
                nc.sync.dma_start(out=b0[:, :],
                                  in_=bT0_d[128 * hp:128 * (hp + 1), :])
                bT0_sb.append(b0)
                b1 = consts.tile([69, 394], dt.bfloat16, name=f"bT1_{hp}",
                                 tag=f"bT1_{hp}")
                nc.sync.dma_start(out=b1[:, :],
                                  in_=bT1_d[69 * hp:69 * (hp + 1), :])
                bT1_sb.append(b1)
            # remaining x chunks
            for c in range(1, nchunks):
                for k in range(6):
                    nc.sync.dma_start(
                        out=xb[k][:, CHUNK * c:CHUNK * (c + 1)],
                        in_=xTb_d[128 * k:128 * (k + 1), CHUNK * c:CHUNK * (c + 1)])
            wp = [wpool.tile([128, DIM], dt.bfloat16, name=f"wp{k}", tag=f"wp{k}")
                  for k in range(6)]
            for k in range(6):
                nc.sync.dma_start(out=wp[k][:, :],
                                  in_=wpT_d[128 * k:128 * (k + 1), :])
            pb_rep = consts.tile([128, DIM], dt.float32, name="pbrep", tag="pbrep")
            nc.sync.dma_start(out=pb_rep[:, :],
                              in_=_ap(pb_d, 0, [[0, 128], [1, DIM]]))

            # ---------- persistent sbuf tiles ----------
            qk_sb = [qkp.tile([128, ntok], dt.bfloat16, name=f"qk{m}", tag=f"qk{m}")
                     for m in range(12)]  # 0-5 q (scaled+biased), 6-11 k
            # v: token-major, 65 cols per head (64 v + shared ones col)
            v_sb = [[vp.tile([n2c, 65 * NH], dt.bfloat16, name=f"v{b}_{ci}",
                             tag=f"v{b}_{ci}")
                     for ci, (n2o, n2c) in enumerate(N1C)] for b in range(nb)]
            ao = [aop.tile([128, ntok], dt.bfloat16, name=f"ao{m}", tag=f"ao{m}")
                  for m in range(6)]  # attn_out^T, head pair per tile

            # ---------- work-unit emitters ----------
            def emit_qk_m(c, m):
                """q/k projection for block m, token chunk c."""
                no = CHUNK * c
                ps = mmps.tile([128, 512], dt.float32, name="mm", tag="mm")
                col = 128 * m if m < 6 else DIM + 128 * (m - 6)
                for k in range(6):
                    nc.tensor.matmul(ps[:, 0:CHUNK],
                                     wqk[k][:, col:col + 128],
                                     xb[k][:, no:no + CHUNK],
                                     start=(k == 0), stop=(k == 5))
                if m < 6:
                    nc.vector.tensor_scalar(
                        out=qk_sb[m][:, no:no + CHUNK], in0=ps[:, 0:CHUNK],
                        scalar1=qb2_sb[:, m:m + 1], scalar2=None,
                        op0=mybir.AluOpType.add)
                else:
                    nc.vector.tensor_copy(qk_sb[m][:, no:no + CHUNK],
                                          ps[:, 0:CHUNK])

            def emit_v(b, ci, half):
                """v projection for batch b, token chunk ci, 384-col half."""
                n2o, n2c = N1C[ci]
                vt = v_sb[b][ci]
                if half == 0:  # ones columns once per tile
                    nc.vector.memset(
                        _ap(vt, vt.offset + 64,
                            [[vt.ap[0][0], n2c], [65, NH]]), 1.0)
                ps = mmps.tile([128, 512], dt.float32, name="mm", tag="mm")
                for k in range(6):
                    nc.tensor.matmul(
                        ps[0:n2c, 0:384],
                        xb[k][:, N_TOK * b + n2o:N_TOK * b + n2o + n2c],
                        wv[k][:, 384 * half:384 * (half + 1)],
                        start=(k == 0), stop=(k == 5))
                nc.vector.tensor_tensor(
                    out=_ap(vt, vt.offset + 65 * 6 * half,
                            [[vt.ap[0][0], n2c], [65, 6], [1, 64]]),
                    in0=ps[0:n2c, 0:384],
                    in1=vb_rep[0:n2c, 384 * half:384 * (half + 1)],
                    op=mybir.AluOpType.add)

            def emit_proj(b, ci, half, ys):
                """proj for batch b, token chunk ci, 384-col half."""
                n2o, n2c = N1C[ci]
                to = N_TOK * b + n2o
                ps = mmps.tile([128, 512], dt.float32, name="mm", tag="mm")
                for k in range(6):
                    nc.tensor.matmul(ps[0:n2c, 0:384],
                                     ao[k][:, to:to + n2c],
                                     wp[k][:, 384 * half:384 * (half + 1)],
                                     start=(k == 0), stop=(k == 5))
                nc.vector.tensor_tensor(
                    out=ys[0:n2c, 384 * half:384 * (half + 1)],
                    in0=ps[0:n2c, 0:384],
                    in1=pb_rep[0:n2c, 384 * half:384 * (half + 1)],
                    op=mybir.AluOpType.add)

            def proj_units(b):
                ys = [ysp.tile([128, DIM], dt.float32, name="ys0", tag="ys0"),
                      ysp.tile([69, DIM], dt.float32, name="ys1", tag="ys1")]
                units = []
                for ci in range(2):
                    for half in range(2):
                        units.append(lambda b=b, ci=ci, half=half:
                                     emit_proj(b, ci, half, ys[ci]))

                    def out_dma(b=b, ci=ci):
                        n2o, n2c = N1C[ci]
                        nc.sync.dma_start(
                            out=y_d[N_TOK * b + n2o:N_TOK * b + n2o + n2c, :],
                            in_=ys[ci][0:n2c, :])
                    units.append(out_dma)
                return units

            # ---------- attention ----------
            SOFF = [0, 256]

            def emit_scores(b, hp):
                """S^T + bias -> exp for head pair hp of batch b.
                One psum tile per head (hi): a psum bank must only ever be
                written by matmuls of a single tile_position mode --
                mixing (0,0) and (64,0) groups in one bank wedges the PE."""
                sph = [mmps.tile([128, 512], dt.float32, name="mm", tag="mm")
                       for _ in range(2)]
                qt = qk_sb[hp]
                kt = qk_sb[6 + hp]
                for hi in range(2):
                    po = 64 * hi
                    for ci, (n2o, n2c) in enumerate(N1C):
                        nc.tensor.matmul(
                            sph[hi][0:n2c, SOFF[ci]:SOFF[ci] + 197],
                            kt[po:po + 64,
                               N_TOK * b + n2o:N_TOK * b + n2o + n2c],
                            qt[po:po + 64, N_TOK * b:N_TOK * (b + 1)],
                            start=True, stop=True)
                ss0 = ss0p.tile([128, 394], dt.float32, name="ss0", tag="ss0")
                ss1 = ss1p.tile([69, 394], dt.float32, name="ss1", tag="ss1")
                ssx = (ss0, ss1)
                bTx = (bT0_sb, bT1_sb)
                for ci, (n2o, n2c) in enumerate(N1C):
                    for hi in range(2):
                        if "bias" in F:
                            nc.vector.tensor_tensor(
                                out=ssx[ci][0:n2c, 197 * hi:197 * (hi + 1)],
                                in0=sph[hi][0:n2c, SOFF[ci]:SOFF[ci] + 197],
                                in1=bTx[ci][hp][0:n2c, 197 * hi:197 * (hi + 1)],
                                op=mybir.AluOpType.add)
                        else:
                            nc.vector.tensor_copy(
                                ssx[ci][0:n2c, 197 * hi:197 * (hi + 1)],
                                sph[hi][0:n2c, SOFF[ci]:SOFF[ci] + 197])
                et0 = et0p.tile([128, 394], dt.bfloat16, name="et0", tag="et0")
                et1 = et1p.tile([69, 394], dt.bfloat16, name="et1", tag="et1")
                if "exp" in F:
                    nc.scalar.activation(out=et0[:, :], in_=ss0[:, :], func=Exp)
                    nc.scalar.activation(out=et1[:, :], in_=ss1[:, :], func=Exp)
                else:
                    nc.vector.tensor_copy(et0[:, :], ss0[:, :])
                    nc.vector.tensor_copy(et1[:, :], ss1[:, :])
                if debug and b == 0 and hp == 0:
                    nc.sync.dma_start(out=dbg_ss0_d[:, :], in_=ss0[:, :])
                    nc.sync.dma_start(out=dbg_ss1_d[:, :], in_=ss1[:, :])
                    nc.sync.dma_start(out=dbg_et0_d[:, :], in_=et0[:, :])
                    nc.sync.dma_start(out=dbg_et1_d[:, :], in_=et1[:, :])
                return et0, et1

            def emit_av(b, hp, et0, et1):
                """AV + denominator + normalize into ao. Broadcast of the
                reciprocal row goes through a DRAM round-trip (0-stride
                partition reads are DRAM-only)."""
                ap_ = avps.tile([128, 512], dt.float32, name="av", tag="av")
                for hi in range(2):
                    h = 2 * hp + hi
                    for ci, (n2o, n2c) in enumerate(N1C):
                        et = (et0, et1)[ci]
                        nc.tensor.matmul(
                            ap_[0:65, 197 * hi:197 * (hi + 1)],
                            v_sb[b][ci][:, 65 * h:65 * (h + 1)],
                            et[0:n2c, 197 * hi:197 * (hi + 1)],
                            start=(ci == 0), stop=(ci == 1))
                if "rt" in F:
                    rc = rcp.tile([65, 394], dt.float32, name="rc", tag="rc")
                    nc.vector.reciprocal(rc[64:65, :], ap_[64:65, 0:394])
                    rd_t = dramp.tile([1, 394], dt.float32, name="rd", tag="rd")
                    nc.sync.dma_start(out=rd_t[0:1, :], in_=rc[64:65, :])
                    rb = rbp.tile([64, 394], dt.float32, name="rb", tag="rb")
                    nc.sync.dma_start(out=rb[0:64, :],
                                      in_=_ap(rd_t, rd_t.offset,
                                              [[0, 64], [1, 394]]))
                    if debug and b == 0 and hp == 0:
                        nc.sync.dma_start(out=dbg_rc_d[0:1, :],
                                          in_=rc[64:65, :])
                        nc.sync.dma_start(out=dbg_rb_d[0:64, :],
                                          in_=rb[0:64, :])
                    for hi in range(2):
                        nc.vector.tensor_tensor(
                            out=ao[hp][64 * hi:64 * (hi + 1),
                                       N_TOK * b:N_TOK * (b + 1)],
                            in0=ap_[0:64, 197 * hi:197 * (hi + 1)],
                            in1=rb[0:64, 197 * hi:197 * (hi + 1)],
                            op=mybir.AluOpType.mult)
                else:
                    for hi in range(2):
                        nc.vector.tensor_copy(
                            ao[hp][64 * hi:64 * (hi + 1),
                                   N_TOK * b:N_TOK * (b + 1)],
                            ap_[0:64, 197 * hi:197 * (hi + 1)])

            # ---------- main schedule ----------
            # prologue: qkv for chunk 0 (batches 0,1), dense
            for m in range(12):
                emit_qk_m(0, m)
            for b in (0, 1):
                for ci in range(2):
                    for half in range(2):
                        emit_v(b, ci, half)

            # per chunk: attention for its 2 batches, with next-chunk qkv and
            # previous-batch proj emitted as PE filler between pair stages.
            for c in range(nchunks):
                filler = []
                if c + 1 < nchunks:
                    filler += [lambda m=m, c=c: emit_qk_m(c + 1, m)
                               for m in range(12)]
                    for b in (2 * (c + 1), 2 * (c + 1) + 1):
                        filler += [lambda b=b, ci=ci, half=half:
                                   emit_v(b, ci, half)
                                   for ci in range(2) for half in range(2)]
                if c >= 1 and "proj" in F:
                    filler += proj_units(2 * (c - 1))
                    filler += proj_units(2 * (c - 1) + 1)
                fit = iter(filler)

                def fill(n=1):
                    if "fill" not in F:
                        return
                    for _ in range(n):
                        u = next(fit, None)
                        if u is not None:
                            u()

                for b in (2 * c, 2 * c + 1):
                    if "scores" not in F:
                        continue
                    pend = []  # (hp, et0, et1) awaiting AV
                    for hp in range(6):
                        ets = emit_scores(b, hp)
                        fill(2)
                        pend.append((hp, ets))
                        if "av" in F and len(pend) >= 2:
                            php, (e0, e1) = pend.pop(0)
                            emit_av(b, php, e0, e1)
                            fill(1)
                    if "av" in F:
                        for php, (e0, e1) in pend:
                            emit_av(b, php, e0, e1)
                            fill(1)
                # drain leftover filler before next chunk's attention
                for u in fit:
                    u()

            # epilogue: proj for the last two batches
            if "proj" in F:
                for u in proj_units(nb - 2):
                    u()
                for u in proj_units(nb - 1):
                    u()

            if debug:
                for m in range(12):
                    nc.sync.dma_start(out=dbg_qk_d[128 * m:128 * (m + 1), :],
                                      in_=qk_sb[m][:, :])
                for ci in range(2):
                    n2c = N1C[ci][1]
                    nc.sync.dma_start(
                        out=dbg_v_d[128 * ci:128 * ci + n2c, :],
                        in_=v_sb[0][ci][:, :])
                for m in range(6):
                    nc.sync.dma_start(out=dbg_ao_d[128 * m:128 * (m + 1), :],
                                      in_=ao[m][:, :])

    nc.compile()
    return nc


def _marshal(x, qkv_w, q_bias, v_bias, rpb_table, proj_w, proj_b, rel_index):
    B = x.shape[0]
    ncore = 8
    bpc = B // ncore

    wqkT = np.ascontiguousarray(qkv_w[0:2 * DIM, :].T.astype(np.float32))
    wqkT[:, 0:DIM] *= SCALE
    wqkT = wqkT.astype(BF16)
    wvT = np.ascontiguousarray(qkv_w[2 * DIM:3 * DIM, :].T.astype(BF16))
    wpT = np.ascontiguousarray(proj_w.T.astype(BF16))
    qb2 = np.ascontiguousarray(
        (q_bias.astype(np.float32) * SCALE).reshape(6, 128).T)

    # full transposed bias, pair-packed: bias[h][n1, n2] -> biasT[h][n2, n1]
    bias = rpb_table[np.asarray(rel_index).reshape(-1)].reshape(
        N_TOK, N_TOK, NH).astype(np.float32)  # [n1, n2, h]
    bT0 = np.zeros((6 * 128, 394), dtype=BF16)
    bT1 = np.zeros((6 * 69, 394), dtype=BF16)
    for hp in range(6):
        for hi in range(2):
            bt = bias[:, :, 2 * hp + hi].T  # [n2, n1]
            bT0[128 * hp:128 * (hp + 1), 197 * hi:197 * (hi + 1)] = bt[0:128, :]
            bT1[69 * hp:69 * (hp + 1), 197 * hi:197 * (hi + 1)] = bt[128:197, :]

    shared = {"wqkT": wqkT, "wvT": wvT, "wpT": wpT, "qb2": qb2,
              "vb": np.ascontiguousarray(v_bias.astype(np.float32)),
              "pb": np.ascontiguousarray(proj_b.astype(np.float32)),
              "bT0": bT0, "bT1": bT1}
    x2 = np.asarray(x, dtype=np.float32).reshape(B, N_TOK, DIM)
    in_maps = []
    for c in range(ncore):
        xTb = np.ascontiguousarray(
            x2[c * bpc:(c + 1) * bpc].reshape(bpc * N_TOK, DIM).T.astype(BF16))
        m = dict(shared)
        m["xTb"] = xTb
        in_maps.append(m)
    return in_maps, bpc


last_exec_time_ns = None
last_results = None


def _install_ntff_hook():
    """Provide antenv.axon_hooks + register the ctypes NTFF hook (the agent
    image's antenv lacks axon_hooks, so trn_boot degraded silently)."""
    import types
    import contextlib
    import ctypes

    try:
        from antenv.axon_hooks import get_axon_ntff_profile_hook
        if get_axon_ntff_profile_hook() is not None:
            return
    except ImportError:
        import antenv
        mod = types.ModuleType("antenv.axon_hooks")
        mod._hook = None

        def set_axon_ntff_profile_hook(h):
            mod._hook = h

        def get_axon_ntff_profile_hook():
            return mod._hook

        mod.set_axon_ntff_profile_hook = set_axon_ntff_profile_hook
        mod.get_axon_ntff_profile_hook = get_axon_ntff_profile_hook
        sys.modules["antenv.axon_hooks"] = mod
        antenv.axon_hooks = mod

    so_path = "/opt/axon/libaxon_pjrt.so"
    lib = ctypes.CDLL(so_path)
    if not hasattr(lib, "axon_start_nrt_profile"):
        return
    lib.axon_start_nrt_profile.argtypes = [ctypes.POINTER(ctypes.c_int64),
                                           ctypes.c_size_t]
    lib.axon_start_nrt_profile.restype = ctypes.c_int64
    lib.axon_stop_nrt_profile.argtypes = [ctypes.c_char_p]
    lib.axon_stop_nrt_profile.restype = ctypes.c_int64

    @contextlib.contextmanager
    def _hook(output_dir, device_ids):
        import jax
        jax.devices()
        if device_ids:
            ids = (ctypes.c_int64 * len(device_ids))(*device_ids)
            rc = lib.axon_start_nrt_profile(ids, len(device_ids))
        else:
            rc = lib.axon_start_nrt_profile(None, 0)
        if rc != 0:
            raise RuntimeError(f"axon_start_nrt_profile rc={rc}")
        try:
            yield
        finally:
            n = lib.axon_stop_nrt_profile(str(output_dir).encode())
            print(f"ntff profile: {n} file(s) -> {output_dir}", file=sys.stderr)

    from antenv.axon_hooks import set_axon_ntff_profile_hook
    set_axon_ntff_profile_hook(_hook)


def kernel(x, qkv_w, q_bias, v_bias, rpb_table, proj_w, proj_b, rel_index):
    global last_exec_time_ns, last_results
    import os
    if os.environ.get("KERNEL_TRACE"):
        _install_ntff_hook()
    from concourse.bass_utils import run_bass_kernel_spmd

    x = np.asarray(x, dtype=np.float32)
    qkv_w = np.asarray(qkv_w, dtype=np.float32)
    q_bias = np.asarray(q_bias, dtype=np.float32)
    v_bias = np.asarray(v_bias, dtype=np.float32)
    rpb_table = np.asarray(rpb_table, dtype=np.float32)
    proj_w = np.asarray(proj_w, dtype=np.float32)
    proj_b = np.asarray(proj_b, dtype=np.float32)

    B = x.shape[0]
    bpc = B // 8
    if 'nc' not in _cache:
        _cache['nc'] = build_program(bpc)
    nc = _cache['nc']

    in_maps, bpc = _marshal(x, qkv_w, q_bias, v_bias, rpb_table,
                            proj_w, proj_b, rel_index)
    res = run_bass_kernel_spmd(nc, in_maps, core_ids=list(range(8)),
                               trace=bool(os.environ.get("KERNEL_TRACE")))
    last_exec_time_ns = res.exec_time_ns
    last_results = res
    ys = [res.results[c]["y"].reshape(bpc, N_TOK, DIM) for c in range(8)]
    return np.concatenate(ys, axis=0).astype(np.float32)


# revision 20
# speedup vs baseline: 1.3080x; 1.1790x over previous
"""TRN2 Bass kernel for BEiT-style attention (nn_Attention_27771258536423).

Strategy: data-parallel over batch across 8 NeuronCores (8 batches/core).
Per core (all matmuls bf16, psum f32):
  - rel-pos bias precomputed on host, shipped TRANSPOSED and pair-packed.
  - qkv: q,k channel-major [ch, tok] bf16 (q pre-scaled+biased via folded
    weights); v token-major [tok, 65*12] bf16 with a ones column per head
    (denominator rides along in the AV matmul).
  - attention per (batch, head-pair): S^T = k^T q directly (no transposes
    anywhere: exp(S^T + bias^T) IS E^T, the AV moving operand).
    AV: out[65, 197] = V_aug^T E^T -> rows 0:64 = attn_out^T (channel-major),
    row 64 = softmax denominator. reciprocal (DVE) -> partition_broadcast
    (gpsimd) -> multiply (DVE) into attn_outT bf16.
  - proj per batch, interleaved into the attention stream as PE filler
    together with next-chunk qkv matmuls (covers softmax latency).
"""
import sys

sys.path.insert(0, '/opt/trn_rl_repo')

import numpy as np
import ml_dtypes

import concourse.bass as bass
import concourse.mybir as mybir
import concourse.tile as tile
from concourse import bacc

dt = mybir.dt
BF16 = ml_dtypes.bfloat16

DIM = 768
NH = 12
HD = 64
N_TOK = 197
SCALE = HD ** (-0.5)
CHUNK = 2 * N_TOK          # 394 tokens = 2 batches per qkv chunk
N1C = [(0, 128), (128, 69)]  # token chunks within one batch (n2 chunks)

_cache = {}


def _ap(t, offset, ap):
    return bass.AP(tensor=t.tensor if hasattr(t, 'tensor') else t,
                   offset=offset, ap=ap)


def build_program(nb, debug=False, feats="scores,bias,exp,av,rt,proj,fill"):
    F = set(feats.split(",")) if feats else set()
    """nb = batches per core (8). Returns compiled Bacc."""
    assert nb % 2 == 0
    ntok = nb * N_TOK
    nchunks = nb // 2

    nc = bacc.Bacc(None)
    if debug:
        dbg_qk_d = nc.dram_tensor("dbg_qk", [12 * 128, ntok], dt.bfloat16,
                                  kind="ExternalOutput")
        dbg_v_d = nc.dram_tensor("dbg_v", [2 * 128, 65 * NH], dt.bfloat16,
                                 kind="ExternalOutput")
        dbg_ss0_d = nc.dram_tensor("dbg_ss0", [128, 394], dt.float32,
                                   kind="ExternalOutput")
        dbg_ss1_d = nc.dram_tensor("dbg_ss1", [69, 394], dt.float32,
                                   kind="ExternalOutput")
        dbg_et0_d = nc.dram_tensor("dbg_et0", [128, 394], dt.bfloat16,
                                   kind="ExternalOutput")
        dbg_et1_d = nc.dram_tensor("dbg_et1", [69, 394], dt.bfloat16,
                                   kind="ExternalOutput")
        dbg_rc_d = nc.dram_tensor("dbg_rc", [1, 394], dt.float32,
                                  kind="ExternalOutput")
        dbg_rb_d = nc.dram_tensor("dbg_rb", [64, 394], dt.float32,
                                  kind="ExternalOutput")
        dbg_ao_d = nc.dram_tensor("dbg_ao", [6 * 128, ntok], dt.bfloat16,
                                  kind="ExternalOutput")

    xTb_d = nc.dram_tensor("xTb", [DIM, ntok], dt.bfloat16, kind="ExternalInput")
    wqkT_d = nc.dram_tensor("wqkT", [DIM, 2 * DIM], dt.bfloat16, kind="ExternalInput")
    wvT_d = nc.dram_tensor("wvT", [DIM, DIM], dt.bfloat16, kind="ExternalInput")
    wpT_d = nc.dram_tensor("wpT", [DIM, DIM], dt.bfloat16, kind="ExternalInput")
    qb2_d = nc.dram_tensor("qb2", [128, 6], dt.float32, kind="ExternalInput")
    vb_d = nc.dram_tensor("vb", [DIM], dt.float32, kind="ExternalInput")
    pb_d = nc.dram_tensor("pb", [DIM], dt.float32, kind="ExternalInput")
    bT0_d = nc.dram_tensor("bT0", [6 * 128, 394], dt.bfloat16, kind="ExternalInput")
    bT1_d = nc.dram_tensor("bT1", [6 * 69, 394], dt.bfloat16, kind="ExternalInput")
    y_d = nc.dram_tensor("y", [ntok, DIM], dt.float32, kind="ExternalOutput")

    Exp = mybir.ActivationFunctionType.Exp

    with tile.TileContext(nc) as tc:
        import contextlib
        with contextlib.ExitStack() as stk:
            consts = stk.enter_context(tc.tile_pool(name="consts", bufs=1))
            wpool = stk.enter_context(tc.tile_pool(name="wpool", bufs=1))
            xp = stk.enter_context(tc.tile_pool(name="xp", bufs=1))
            qkp = stk.enter_context(tc.tile_pool(name="qkp", bufs=1))
            vp = stk.enter_context(tc.tile_pool(name="vp", bufs=1))
            aop = stk.enter_context(tc.tile_pool(name="aop", bufs=1))
            ss0p = stk.enter_context(tc.tile_pool(name="ss0p", bufs=3))
            ss1p = stk.enter_context(tc.tile_pool(name="ss1p", bufs=3))
            et0p = stk.enter_context(tc.tile_pool(name="et0p", bufs=3))
            et1p = stk.enter_context(tc.tile_pool(name="et1p", bufs=3))
            rcp = stk.enter_context(tc.tile_pool(name="rcp", bufs=4))
            rbp = stk.enter_context(tc.tile_pool(name="rbp", bufs=4))
            ysp = stk.enter_context(tc.tile_pool(name="ysp", bufs=3))
            mmps = stk.enter_context(tc.tile_pool(name="mmps", bufs=5, space="PSUM"))
            avps = stk.enter_context(tc.tile_pool(name="avps", bufs=3, space="PSUM"))
            dramp = stk.enter_context(tc.tile_pool(name="dramp", bufs=5, space="DRAM"))

            # ---------- constant / weight DMAs ----------
            # x chunk 0 + q-weight tiles first (first matmuls need them)
            xb = [xp.tile([128, ntok], dt.bfloat16, name=f"xb{k}", tag=f"xb{k}")
                  for k in range(6)]
            for k in range(6):
                nc.sync.dma_start(out=xb[k][:, 0:CHUNK],
                                  in_=xTb_d[128 * k:128 * (k + 1), 0:CHUNK])
            wqk = [wpool.tile([128, 2 * DIM], dt.bfloat16, name=f"wqk{k}",
                              tag=f"wqk{k}") for k in range(6)]
            for k in range(6):   # q columns first
                nc.sync.dma_start(out=wqk[k][:, 0:DIM],
                                  in_=wqkT_d[128 * k:128 * (k + 1), 0:DIM])
            qb2_sb = consts.tile([128, 6], dt.float32, name="qb2", tag="qb2")
            nc.sync.dma_start(out=qb2_sb[:, :], in_=qb2_d[:, :])
            for k in range(6):   # k columns
                nc.sync.dma_start(out=wqk[k][:, DIM:2 * DIM],
                                  in_=wqkT_d[128 * k:128 * (k + 1), DIM:2 * DIM])
            wv = [wpool.tile([128, DIM], dt.bfloat16, name=f"wv{k}", tag=f"wv{k}")
                  for k in range(6)]
            for k in range(6):
                nc.sync.dma_start(out=wv[k][:, :],
                                  in_=wvT_d[128 * k:128 * (k + 1), :])
            vb_rep = consts.tile([128, DIM], dt.float32, name="vbrep", tag="vbrep")
            nc.sync.dma_start(out=vb_rep[:, :],
                              in_=_ap(vb_d, 0, [[0, 128], [1, DIM]]))
            # bias tiles (needed at attention time)
            bT0_sb = []
            bT1_sb = []
            for hp in range(6):
                b0 = consts.tile([128, 394], dt.bfloat16, name=f"bT0_{hp}",
                                 tag=f"bT0_{hp}")
                nc.sync.dma_start(out=b0[:, :],
                                  in_=bT0_d[128 * hp:128 * (hp + 1), :])
                bT0_sb.append(b0)
                b1 = consts.tile([69, 394], dt.bfloat16, name=f"bT1_{hp}",
                                 tag=f"bT1_{hp}")
                nc.sync.dma_start(out=b1[:, :],
                                  in_=bT1_d[69 * hp:69 * (hp + 1), :])
                bT1_sb.append(b1)
            # remaining x chunks
            for c in range(1, nchunks):
                for k in range(6):
                    nc.sync.dma_start(
                        out=xb[k][:, CHUNK * c:CHUNK * (c + 1)],
                        in_=xTb_d[128 * k:128 * (k + 1), CHUNK * c:CHUNK * (c + 1)])
            wp = [wpool.tile([128, DIM], dt.bfloat16, name=f"wp{k}", tag=f"wp{k}")
                  for k in range(6)]
            for k in range(6):
                nc.sync.dma_start(out=wp[k][:, :],
                                  in_=wpT_d[128 * k:128 * (k + 1), :])
            pb_rep = consts.tile([128, DIM], dt.float32, name="pbrep", tag="pbrep")
            nc.sync.dma_start(out=pb_rep[:, :],
                              in_=_ap(pb_d, 0, [[0, 128], [1, DIM]]))

            # ---------- persistent sbuf tiles ----------
            qk_sb = [qkp.tile([128, ntok], dt.bfloat16, name=f"qk{m}", tag=f"qk{m}")
                     for m in range(12)]  # 0-5 q (scaled+biased), 6-11 k
            # v: token-major, 65 cols per head (64 v + shared ones col)
            v_sb = [[vp.tile([n2c, 65 * NH], dt.bfloat16, name=f"v{b}_{ci}",
                             tag=f"v{b}_{ci}")
                     for ci, (n2o, n2c) in enumerate(N1C)] for b in range(nb)]
            ao = [aop.tile([128, ntok], dt.bfloat16, name=f"ao{m}", tag=f"ao{m}")
                  for m in range(6)]  # attn_out^T, head pair per tile

            # ---------- work-unit emitters ----------
            def emit_qk_m(c, m):
                """q/k projection for block m, token chunk c."""
                no = CHUNK * c
                ps = mmps.tile([128, 512], dt.float32, name="mm", tag="mm")
                col = 128 * m if m < 6 else DIM + 128 * (m - 6)
                for k in range(6):
                    nc.tensor.matmul(ps[:, 0:CHUNK],
                                     wqk[k][:, col:col + 128],
                                     xb[k][:, no:no + CHUNK],
                                     start=(k == 0), stop=(k == 5))
                if m < 6:
                    nc.vector.tensor_scalar(
                        out=qk_sb[m][:, no:no + CHUNK], in0=ps[:, 0:CHUNK],
                        scalar1=qb2_sb[:, m:m + 1], scalar2=None,
                        op0=mybir.AluOpType.add)
                else:
                    nc.vector.tensor_copy(qk_sb[m][:, no:no + CHUNK],
                                          ps[:, 0:CHUNK])

            def emit_v(b, ci, half):
                """v projection for batch b, token chunk ci, 384-col half."""
                n2o, n2c = N1C[ci]
                vt = v_sb[b][ci]
                if half == 0:  # ones columns once per tile
                    nc.vector.memset(
                        _ap(vt, vt.offset + 64,
                            [[vt.ap[0][0], n2c], [65, NH]]), 1.0)
                ps = mmps.tile([128, 512], dt.float32, name="mm", tag="mm")
                for k in range(6):
                    nc.tensor.matmul(
                        ps[0:n2c, 0:384],
                        xb[k][:, N_TOK * b + n2o:N_TOK * b + n2o + n2c],
                        wv[k][:, 384 * half:384 * (half + 1)],
                        start=(k == 0), stop=(k == 5))
                nc.vector.tensor_tensor(
                    out=_ap(vt, vt.offset + 65 * 6 * half,
                            [[vt.ap[0][0], n2c], [65, 6], [1, 64]]),
                    in0=ps[0:n2c, 0:384],
                    in1=vb_rep[0:n2c, 384 * half:384 * (half + 1)],
                    op=mybir.AluOpType.add)

            def emit_proj(b, ci, half, ys):
                """proj for batch b, token chunk ci, 384-col half."""
                n2o, n2c = N1C[ci]
                to = N_TOK * b + n2o
                ps = mmps.tile([128, 512], dt.float32, name="mm", tag="mm")
                for k in range(6):
                    nc.tensor.matmul(ps[0:n2c, 0:384],
                                     ao[k][:, to:to + n2c],
                                     wp[k][:, 384 * half:384 * (half + 1)],
                                     start=(k == 0), stop=(k == 5))
                nc.vector.tensor_tensor(
                    out=ys[0:n2c, 384 * half:384 * (half + 1)],
                    in0=ps[0:n2c, 0:384],
                    in1=pb_rep[0:n2c, 384 * half:384 * (half + 1)],
                    op=mybir.AluOpType.add)

            def proj_units(b):
                ys = [ysp.tile([128, DIM], dt.float32, name="ys0", tag="ys0"),
                      ysp.tile([69, DIM], dt.float32, name="ys1", tag="ys1")]
                units = []
                for ci in range(2):
                    for half in range(2):
                        units.append(lambda b=b, ci=ci, half=half:
                                     emit_proj(b, ci, half, ys[ci]))

                    def out_dma(b=b, ci=ci):
                        n2o, n2c = N1C[ci]
                        nc.sync.dma_start(
                            out=y_d[N_TOK * b + n2o:N_TOK * b + n2o + n2c, :],
                            in_=ys[ci][0:n2c, :])
                    units.append(out_dma)
                return units

            # ---------- attention ----------
            SOFF = [0, 256]

            def emit_scores(b, hp):
                """S^T + bias -> exp for head pair hp of batch b.
                One psum tile per head (hi): a psum bank must only ever be
                written by matmuls of a single tile_position mode --
                mixing (0,0) and (64,0) groups in one bank wedges the PE."""
                sph = [mmps.tile([128, 512], dt.float32, name="mm", tag="mm")
                       for _ in range(2)]
                qt = qk_sb[hp]
                kt = qk_sb[6 + hp]
                for hi in range(2):
                    po = 64 * hi
                    for ci, (n2o, n2c) in enumerate(N1C):
                        nc.tensor.matmul(
                            sph[hi][0:n2c, SOFF[ci]:SOFF[ci] + 197],
                            kt[po:po + 64,
                               N_TOK * b + n2o:N_TOK * b + n2o + n2c],
                            qt[po:po + 64, N_TOK * b:N_TOK * (b + 1)],
                            start=True, stop=True)
                ss0 = ss0p.tile([128, 394], dt.float32, name="ss0", tag="ss0")
                ss1 = ss1p.tile([69, 394], dt.float32, name="ss1", tag="ss1")
                ssx = (ss0, ss1)
                bTx = (bT0_sb, bT1_sb)
                for ci, (n2o, n2c) in enumerate(N1C):
                    for hi in range(2):
                        if "bias" in F:
                            nc.vector.tensor_tensor(
                                out=ssx[ci][0:n2c, 197 * hi:197 * (hi + 1)],
                                in0=sph[hi][0:n2c, SOFF[ci]:SOFF[ci] + 197],
                                in1=bTx[ci][hp][0:n2c, 197 * hi:197 * (hi + 1)],
                                op=mybir.AluOpType.add)
                        else:
                            nc.vector.tensor_copy(
                                ssx[ci][0:n2c, 197 * hi:197 * (hi + 1)],
                                sph[hi][0:n2c, SOFF[ci]:SOFF[ci] + 197])
                et0 = et0p.tile([128, 394], dt.bfloat16, name="et0", tag="et0")
                et1 = et1p.tile([69, 394], dt.bfloat16, name="et1", tag="et1")
                if "exp" in F:
                    nc.scalar.activation(out=et0[:, :], in_=ss0[:, :], func=Exp)
                    nc.scalar.activation(out=et1[:, :], in_=ss1[:, :], func=Exp)
                else:
                    nc.vector.tensor_copy(et0[:, :], ss0[:, :])
                    nc.vector.tensor_copy(et1[:, :], ss1[:, :])
                if debug and b == 0 and hp == 0:
                    nc.sync.dma_start(out=dbg_ss0_d[:, :], in_=ss0[:, :])
                    nc.sync.dma_start(out=dbg_ss1_d[:, :], in_=ss1[:, :])
                    nc.sync.dma_start(out=dbg_et0_d[:, :], in_=et0[:, :])
                    nc.sync.dma_start(out=dbg_et1_d[:, :], in_=et1[:, :])
                return et0, et1

            def emit_av(b, hp, et0, et1):
                """AV + denominator + normalize into ao. Broadcast of the
                reciprocal row goes through a DRAM round-trip (0-stride
                partition reads are DRAM-only)."""
                ap_ = avps.tile([128, 512], dt.float32, name="av", tag="av")
                for hi in range(2):
                    h = 2 * hp + hi
                    for ci, (n2o, n2c) in enumerate(N1C):
                        et = (et0, et1)[ci]
                        nc.tensor.matmul(
                            ap_[0:65, 197 * hi:197 * (hi + 1)],
                            v_sb[b][ci][:, 65 * h:65 * (h + 1)],
                            et[0:n2c, 197 * hi:197 * (hi + 1)],
                            start=(ci == 0), stop=(ci == 1))
                if "rt" in F:
                    # numerator + denom row out of psum (frees the bank fast)
                    anum = rcp.tile([65, 394], dt.float32, name="rc", tag="rc")
                    nc.vector.tensor_copy(anum[0:64, :], ap_[0:64, 0:394])
                    nc.vector.tensor_copy(anum[64:65, :], ap_[64:65, 0:394])
                    # reciprocal via DRAM wrap: [1,394] -> [99,4] so the DVE
                    # reciprocal (slow per-lane) runs on 99 lanes, not 1
                    rd_t = dramp.tile([1, 396], dt.float32, name="rd", tag="rd")
                    nc.sync.dma_start(out=rd_t[0:1, 0:394], in_=anum[64:65, :])
                    nc.sync.dma_start(out=rd_t[0:1, 394:396],
                                      in_=anum[64:65, 0:2])
                    rw = rbp.tile([99, 4], dt.float32, name="rw", tag="rw")
                    nc.sync.dma_start(out=rw[0:99, :],
                                      in_=_ap(rd_t, rd_t.offset,
                                              [[4, 99], [1, 4]]))
                    rwr = rbp.tile([99, 4], dt.float32, name="rwr", tag="rwr")
                    nc.vector.reciprocal(rwr[0:99, :], rw[0:99, :])
                    rd2_t = dramp.tile([1, 396], dt.float32, name="rd2",
                                       tag="rd2")
                    nc.sync.dma_start(out=_ap(rd2_t, rd2_t.offset,
                                              [[4, 99], [1, 4]]),
                                      in_=rwr[0:99, :])
                    rb = rbp.tile([64, 394], dt.float32, name="rb", tag="rb")
                    nc.sync.dma_start(out=rb[0:64, :],
                                      in_=_ap(rd2_t, rd2_t.offset,
                                              [[0, 64], [1, 394]]))
                    if debug and b == 0 and hp == 0:
                        nc.sync.dma_start(out=dbg_rc_d[0:1, :],
                                          in_=_ap(rd2_t, rd2_t.offset,
                                                  [[0, 1], [1, 394]]))
                        nc.sync.dma_start(out=dbg_rb_d[0:64, :],
                                          in_=rb[0:64, :])
                    for hi in range(2):
                        nc.vector.tensor_tensor(
                            out=ao[hp][64 * hi:64 * (hi + 1),
                                       N_TOK * b:N_TOK * (b + 1)],
                            in0=anum[0:64, 197 * hi:197 * (hi + 1)],
                            in1=rb[0:64, 197 * hi:197 * (hi + 1)],
                            op=mybir.AluOpType.mult)
                else:
                    for hi in range(2):
                        nc.vector.tensor_copy(
                            ao[hp][64 * hi:64 * (hi + 1),
                                   N_TOK * b:N_TOK * (b + 1)],
                            ap_[0:64, 197 * hi:197 * (hi + 1)])

            # ---------- main schedule ----------
            # prologue: qkv for chunk 0 (batches 0,1), dense
            for m in range(12):
                emit_qk_m(0, m)
            for b in (0, 1):
                for ci in range(2):
                    for half in range(2):
                        emit_v(b, ci, half)

            # per chunk: attention for its 2 batches, with next-chunk qkv and
            # previous-batch proj emitted as PE filler between pair stages.
            for c in range(nchunks):
                filler = []
                if c + 1 < nchunks:
                    filler += [lambda m=m, c=c: emit_qk_m(c + 1, m)
                               for m in range(12)]
                    for b in (2 * (c + 1), 2 * (c + 1) + 1):
                        filler += [lambda b=b, ci=ci, half=half:
                                   emit_v(b, ci, half)
                                   for ci in range(2) for half in range(2)]
                if c >= 1 and "proj" in F:
                    filler += proj_units(2 * (c - 1))
                    filler += proj_units(2 * (c - 1) + 1)
                fit = iter(filler)

                def fill(n=1):
                    if "fill" not in F:
                        return
                    for _ in range(n):
                        u = next(fit, None)
                        if u is not None:
                            u()

                for b in (2 * c, 2 * c + 1):
                    if "scores" not in F:
                        continue
                    pend = []  # (hp, et0, et1) awaiting AV
                    for hp in range(6):
                        ets = emit_scores(b, hp)
                        fill(2)
                        pend.append((hp, ets))
                        if "av" in F and len(pend) >= 2:
                            php, (e0, e1) = pend.pop(0)
                            emit_av(b, php, e0, e1)
                            fill(1)
                    if "av" in F:
                        for php, (e0, e1) in pend:
                            emit_av(b, php, e0, e1)
                            fill(1)
                # drain leftover filler before next chunk's attention
                for u in fit:
                    u()

            # epilogue: proj for the last two batches
            if "proj" in F:
                for u in proj_units(nb - 2):
                    u()
                for u in proj_units(nb - 1):
                    u()

            if debug:
                for m in range(12):
                    nc.sync.dma_start(out=dbg_qk_d[128 * m:128 * (m + 1), :],
                                      in_=qk_sb[m][:, :])
                for ci in range(2):
                    n2c = N1C[ci][1]
                    nc.sync.dma_start(
                        out=dbg_v_d[128 * ci:128 * ci + n2c, :],
                        in_=v_sb[0][ci][:, :])
                for m in range(6):
                    nc.sync.dma_start(out=dbg_ao_d[128 * m:128 * (m + 1), :],
                                      in_=ao[m][:, :])

    nc.compile()
    return nc


def _marshal(x, qkv_w, q_bias, v_bias, rpb_table, proj_w, proj_b, rel_index):
    B = x.shape[0]
    ncore = 8
    bpc = B // ncore

    wqkT = np.ascontiguousarray(qkv_w[0:2 * DIM, :].T.astype(np.float32))
    wqkT[:, 0:DIM] *= SCALE
    wqkT = wqkT.astype(BF16)
    wvT = np.ascontiguousarray(qkv_w[2 * DIM:3 * DIM, :].T.astype(BF16))
    wpT = np.ascontiguousarray(proj_w.T.astype(BF16))
    qb2 = np.ascontiguousarray(
        (q_bias.astype(np.float32) * SCALE).reshape(6, 128).T)

    # full transposed bias, pair-packed: bias[h][n1, n2] -> biasT[h][n2, n1]
    bias = rpb_table[np.asarray(rel_index).reshape(-1)].reshape(
        N_TOK, N_TOK, NH).astype(np.float32)  # [n1, n2, h]
    bT0 = np.zeros((6 * 128, 394), dtype=BF16)
    bT1 = np.zeros((6 * 69, 394), dtype=BF16)
    for hp in range(6):
        for hi in range(2):
            bt = bias[:, :, 2 * hp + hi].T  # [n2, n1]
            bT0[128 * hp:128 * (hp + 1), 197 * hi:197 * (hi + 1)] = bt[0:128, :]
            bT1[69 * hp:69 * (hp + 1), 197 * hi:197 * (hi + 1)] = bt[128:197, :]

    shared = {"wqkT": wqkT, "wvT": wvT, "wpT": wpT, "qb2": qb2,
              "vb": np.ascontiguousarray(v_bias.astype(np.float32)),
              "pb": np.ascontiguousarray(proj_b.astype(np.float32)),
              "bT0": bT0, "bT1": bT1}
    x2 = np.asarray(x, dtype=np.float32).reshape(B, N_TOK, DIM)
    in_maps = []
    for c in range(ncore):
        xTb = np.ascontiguousarray(
            x2[c * bpc:(c + 1) * bpc].reshape(bpc * N_TOK, DIM).T.astype(BF16))
        m = dict(shared)
        m["xTb"] = xTb
        in_maps.append(m)
    return in_maps, bpc


last_exec_time_ns = None
last_results = None


def _install_ntff_hook():
    """Provide antenv.axon_hooks + register the ctypes NTFF hook (the agent
    image's antenv lacks axon_hooks, so trn_boot degraded silently)."""
    import types
    import contextlib
    import ctypes

    try:
        from antenv.axon_hooks import get_axon_ntff_profile_hook
        if get_axon_ntff_profile_hook() is not None:
            return
    except ImportError:
        import antenv
        mod = types.ModuleType("antenv.axon_hooks")
        mod._hook = None

        def set_axon_ntff_profile_hook(h):
            mod._hook = h

        def get_axon_ntff_profile_hook():
            return mod._hook

        mod.set_axon_ntff_profile_hook = set_axon_ntff_profile_hook
        mod.get_axon_ntff_profile_hook = get_axon_ntff_profile_hook
        sys.modules["antenv.axon_hooks"] = mod
        antenv.axon_hooks = mod

    so_path = "/opt/axon/libaxon_pjrt.so"
    lib = ctypes.CDLL(so_path)
    if not hasattr(lib, "axon_start_nrt_profile"):
        return
    lib.axon_start_nrt_profile.argtypes = [ctypes.POINTER(ctypes.c_int64),
                                           ctypes.c_size_t]
    lib.axon_start_nrt_profile.restype = ctypes.c_int64
    lib.axon_stop_nrt_profile.argtypes = [ctypes.c_char_p]
    lib.axon_stop_nrt_profile.restype = ctypes.c_int64

    @contextlib.contextmanager
    def _hook(output_dir, device_ids):
        import jax
        jax.devices()
        if device_ids:
            ids = (ctypes.c_int64 * len(device_ids))(*device_ids)
            rc = lib.axon_start_nrt_profile(ids, len(device_ids))
        else:
            rc = lib.axon_start_nrt_profile(None, 0)
        if rc != 0:
            raise RuntimeError(f"axon_start_nrt_profile rc={rc}")
        try:
            yield
        finally:
            n = lib.axon_stop_nrt_profile(str(output_dir).encode())
            print(f"ntff profile: {n} file(s) -> {output_dir}", file=sys.stderr)

    from antenv.axon_hooks import set_axon_ntff_profile_hook
    set_axon_ntff_profile_hook(_hook)


def kernel(x, qkv_w, q_bias, v_bias, rpb_table, proj_w, proj_b, rel_index):
    global last_exec_time_ns, last_results
    import os
    if os.environ.get("KERNEL_TRACE"):
        _install_ntff_hook()
    from concourse.bass_utils import run_bass_kernel_spmd

    x = np.asarray(x, dtype=np.float32)
    qkv_w = np.asarray(qkv_w, dtype=np.float32)
    q_bias = np.asarray(q_bias, dtype=np.float32)
    v_bias = np.asarray(v_bias, dtype=np.float32)
    rpb_table = np.asarray(rpb_table, dtype=np.float32)
    proj_w = np.asarray(proj_w, dtype=np.float32)
    proj_b = np.asarray(proj_b, dtype=np.float32)

    B = x.shape[0]
    bpc = B // 8
    if 'nc' not in _cache:
        _cache['nc'] = build_program(bpc)
    nc = _cache['nc']

    in_maps, bpc = _marshal(x, qkv_w, q_bias, v_bias, rpb_table,
                            proj_w, proj_b, rel_index)
    res = run_bass_kernel_spmd(nc, in_maps, core_ids=list(range(8)),
                               trace=bool(os.environ.get("KERNEL_TRACE")))
    last_exec_time_ns = res.exec_time_ns
    last_results = res
    ys = [res.results[c]["y"].reshape(bpc, N_TOK, DIM) for c in range(8)]
    return np.concatenate(ys, axis=0).astype(np.float32)


# revision 21
# speedup vs baseline: 1.5580x; 1.1911x over previous
"""TRN2 Bass kernel for BEiT-style attention (nn_Attention_27771258536423).

Strategy: data-parallel over batch across 8 NeuronCores (8 batches/core).
Per core (all matmuls bf16, psum f32):
  - rel-pos bias precomputed on host, shipped TRANSPOSED and pair-packed.
  - qkv: q,k channel-major [ch, tok] bf16 (q pre-scaled+biased via folded
    weights); v token-major [tok, 65*12] bf16 with a ones column per head
    (denominator rides along in the AV matmul).
  - attention per (batch, head-pair): S^T = k^T q directly (no transposes
    anywhere: exp(S^T + bias^T) IS E^T, the AV moving operand).
    AV: out[65, 197] = V_aug^T E^T -> rows 0:64 = attn_out^T (channel-major),
    row 64 = softmax denominator. reciprocal (DVE) -> partition_broadcast
    (gpsimd) -> multiply (DVE) into attn_outT bf16.
  - proj per batch, interleaved into the attention stream as PE filler
    together with next-chunk qkv matmuls (covers softmax latency).
"""
import sys

sys.path.insert(0, '/opt/trn_rl_repo')

import numpy as np
import ml_dtypes

import concourse.bass as bass
import concourse.mybir as mybir
import concourse.tile as tile
from concourse import bacc

dt = mybir.dt
BF16 = ml_dtypes.bfloat16

DIM = 768
NH = 12
HD = 64
N_TOK = 197
SCALE = HD ** (-0.5)
CHUNK = 2 * N_TOK          # 394 tokens = 2 batches per qkv chunk
N1C = [(0, 128), (128, 69)]  # token chunks within one batch (n2 chunks)

_cache = {}


def _ap(t, offset, ap):
    return bass.AP(tensor=t.tensor if hasattr(t, 'tensor') else t,
                   offset=offset, ap=ap)


def build_program(nb, debug=False, feats="scores,bias,exp,av,rt,proj,fill"):
    F = set(feats.split(",")) if feats else set()
    """nb = batches per core (8). Returns compiled Bacc."""
    assert nb % 2 == 0
    ntok = nb * N_TOK
    nchunks = nb // 2

    nc = bacc.Bacc(None)
    if debug:
        dbg_qk_d = nc.dram_tensor("dbg_qk", [12 * 128, ntok], dt.bfloat16,
                                  kind="ExternalOutput")
        dbg_v_d = nc.dram_tensor("dbg_v", [2 * 128, 65 * NH], dt.bfloat16,
                                 kind="ExternalOutput")
        dbg_ss0_d = nc.dram_tensor("dbg_ss0", [128, 394], dt.float32,
                                   kind="ExternalOutput")
        dbg_ss1_d = nc.dram_tensor("dbg_ss1", [69, 394], dt.float32,
                                   kind="ExternalOutput")
        dbg_et0_d = nc.dram_tensor("dbg_et0", [128, 394], dt.bfloat16,
                                   kind="ExternalOutput")
        dbg_et1_d = nc.dram_tensor("dbg_et1", [69, 394], dt.bfloat16,
                                   kind="ExternalOutput")
        dbg_rc_d = nc.dram_tensor("dbg_rc", [1, 394], dt.float32,
                                  kind="ExternalOutput")
        dbg_rb_d = nc.dram_tensor("dbg_rb", [64, 394], dt.float32,
                                  kind="ExternalOutput")
        dbg_ao_d = nc.dram_tensor("dbg_ao", [6 * 128, ntok], dt.bfloat16,
                                  kind="ExternalOutput")

    xTb_d = nc.dram_tensor("xTb", [DIM, ntok], dt.bfloat16, kind="ExternalInput")
    wqkT_d = nc.dram_tensor("wqkT", [DIM, 2 * DIM], dt.bfloat16, kind="ExternalInput")
    wvT_d = nc.dram_tensor("wvT", [DIM, DIM], dt.bfloat16, kind="ExternalInput")
    wpT_d = nc.dram_tensor("wpT", [DIM, DIM], dt.bfloat16, kind="ExternalInput")
    qb2_d = nc.dram_tensor("qb2", [128, 6], dt.float32, kind="ExternalInput")
    vb_d = nc.dram_tensor("vb", [DIM], dt.float32, kind="ExternalInput")
    pb_d = nc.dram_tensor("pb", [DIM], dt.float32, kind="ExternalInput")
    bT0_d = nc.dram_tensor("bT0", [6 * 128, 394], dt.bfloat16, kind="ExternalInput")
    bT1_d = nc.dram_tensor("bT1", [6 * 69, 394], dt.bfloat16, kind="ExternalInput")
    y_d = nc.dram_tensor("y", [ntok, DIM], dt.float32, kind="ExternalOutput")

    Exp = mybir.ActivationFunctionType.Exp

    with tile.TileContext(nc) as tc:
        import contextlib
        with contextlib.ExitStack() as stk:
            consts = stk.enter_context(tc.tile_pool(name="consts", bufs=1))
            wpool = stk.enter_context(tc.tile_pool(name="wpool", bufs=1))
            xp = stk.enter_context(tc.tile_pool(name="xp", bufs=1))
            qkp = stk.enter_context(tc.tile_pool(name="qkp", bufs=1))
            vp = stk.enter_context(tc.tile_pool(name="vp", bufs=1))
            aop = stk.enter_context(tc.tile_pool(name="aop", bufs=1))
            ss0p = stk.enter_context(tc.tile_pool(name="ss0p", bufs=3))
            ss1p = stk.enter_context(tc.tile_pool(name="ss1p", bufs=3))
            et0p = stk.enter_context(tc.tile_pool(name="et0p", bufs=3))
            et1p = stk.enter_context(tc.tile_pool(name="et1p", bufs=3))
            rcp = stk.enter_context(tc.tile_pool(name="rcp", bufs=4))
            rbp = stk.enter_context(tc.tile_pool(name="rbp", bufs=8))
            anump = stk.enter_context(tc.tile_pool(name="anump", bufs=8))
            rwp = stk.enter_context(tc.tile_pool(name="rwp", bufs=2))
            ysp = stk.enter_context(tc.tile_pool(name="ysp", bufs=3))
            mmps = stk.enter_context(tc.tile_pool(name="mmps", bufs=5, space="PSUM"))
            avps = stk.enter_context(tc.tile_pool(name="avps", bufs=3, space="PSUM"))
            dramp = stk.enter_context(tc.tile_pool(name="dramp", bufs=2, space="DRAM"))

            # ---------- constant / weight DMAs ----------
            # x chunk 0 + q-weight tiles first (first matmuls need them)
            xb = [xp.tile([128, ntok], dt.bfloat16, name=f"xb{k}", tag=f"xb{k}")
                  for k in range(6)]
            for k in range(6):
                nc.sync.dma_start(out=xb[k][:, 0:CHUNK],
                                  in_=xTb_d[128 * k:128 * (k + 1), 0:CHUNK])
            wqk = [wpool.tile([128, 2 * DIM], dt.bfloat16, name=f"wqk{k}",
                              tag=f"wqk{k}") for k in range(6)]
            for k in range(6):   # q columns first
                nc.sync.dma_start(out=wqk[k][:, 0:DIM],
                                  in_=wqkT_d[128 * k:128 * (k + 1), 0:DIM])
            qb2_sb = consts.tile([128, 6], dt.float32, name="qb2", tag="qb2")
            nc.sync.dma_start(out=qb2_sb[:, :], in_=qb2_d[:, :])
            for k in range(6):   # k columns
                nc.sync.dma_start(out=wqk[k][:, DIM:2 * DIM],
                                  in_=wqkT_d[128 * k:128 * (k + 1), DIM:2 * DIM])
            wv = [wpool.tile([128, DIM], dt.bfloat16, name=f"wv{k}", tag=f"wv{k}")
                  for k in range(6)]
            for k in range(6):
                nc.sync.dma_start(out=wv[k][:, :],
                                  in_=wvT_d[128 * k:128 * (k + 1), :])
            vb_rep = consts.tile([128, DIM], dt.float32, name="vbrep", tag="vbrep")
            nc.sync.dma_start(out=vb_rep[:, :],
                              in_=_ap(vb_d, 0, [[0, 128], [1, DIM]]))
            # bias tiles (needed at attention time)
            bT0_sb = []
            bT1_sb = []
            for hp in range(6):
                b0 = consts.tile([128, 394], dt.bfloat16, name=f"bT0_{hp}",
                                 tag=f"bT0_{hp}")
                nc.sync.dma_start(out=b0[:, :],
                                  in_=bT0_d[128 * hp:128 * (hp + 1), :])
                bT0_sb.append(b0)
                b1 = consts.tile([69, 394], dt.bfloat16, name=f"bT1_{hp}",
                                 tag=f"bT1_{hp}")
                nc.sync.dma_start(out=b1[:, :],
                                  in_=bT1_d[69 * hp:69 * (hp + 1), :])
                bT1_sb.append(b1)
            # remaining x chunks
            for c in range(1, nchunks):
                for k in range(6):
                    nc.sync.dma_start(
                        out=xb[k][:, CHUNK * c:CHUNK * (c + 1)],
                        in_=xTb_d[128 * k:128 * (k + 1), CHUNK * c:CHUNK * (c + 1)])
            wp = [wpool.tile([128, DIM], dt.bfloat16, name=f"wp{k}", tag=f"wp{k}")
                  for k in range(6)]
            for k in range(6):
                nc.sync.dma_start(out=wp[k][:, :],
                                  in_=wpT_d[128 * k:128 * (k + 1), :])
            pb_rep = consts.tile([128, DIM], dt.float32, name="pbrep", tag="pbrep")
            nc.sync.dma_start(out=pb_rep[:, :],
                              in_=_ap(pb_d, 0, [[0, 128], [1, DIM]]))

            # ---------- persistent sbuf tiles ----------
            qk_sb = [qkp.tile([128, ntok], dt.bfloat16, name=f"qk{m}", tag=f"qk{m}")
                     for m in range(12)]  # 0-5 q (scaled+biased), 6-11 k
            # v: token-major, 65 cols per head (64 v + shared ones col)
            v_sb = [[vp.tile([n2c, 65 * NH], dt.bfloat16, name=f"v{b}_{ci}",
                             tag=f"v{b}_{ci}")
                     for ci, (n2o, n2c) in enumerate(N1C)] for b in range(nb)]
            ao = [aop.tile([128, ntok], dt.bfloat16, name=f"ao{m}", tag=f"ao{m}")
                  for m in range(6)]  # attn_out^T, head pair per tile

            # ---------- work-unit emitters ----------
            def emit_qk_m(c, m):
                """q/k projection for block m, token chunk c."""
                no = CHUNK * c
                ps = mmps.tile([128, 512], dt.float32, name="mm", tag="mm")
                col = 128 * m if m < 6 else DIM + 128 * (m - 6)
                for k in range(6):
                    nc.tensor.matmul(ps[:, 0:CHUNK],
                                     wqk[k][:, col:col + 128],
                                     xb[k][:, no:no + CHUNK],
                                     start=(k == 0), stop=(k == 5))
                if m < 6:
                    nc.vector.tensor_scalar(
                        out=qk_sb[m][:, no:no + CHUNK], in0=ps[:, 0:CHUNK],
                        scalar1=qb2_sb[:, m:m + 1], scalar2=None,
                        op0=mybir.AluOpType.add)
                else:
                    nc.vector.tensor_copy(qk_sb[m][:, no:no + CHUNK],
                                          ps[:, 0:CHUNK])

            def emit_v(b, ci, half):
                """v projection for batch b, token chunk ci, 384-col half."""
                n2o, n2c = N1C[ci]
                vt = v_sb[b][ci]
                if half == 0:  # ones columns once per tile
                    nc.vector.memset(
                        _ap(vt, vt.offset + 64,
                            [[vt.ap[0][0], n2c], [65, NH]]), 1.0)
                ps = mmps.tile([128, 512], dt.float32, name="mm", tag="mm")
                for k in range(6):
                    nc.tensor.matmul(
                        ps[0:n2c, 0:384],
                        xb[k][:, N_TOK * b + n2o:N_TOK * b + n2o + n2c],
                        wv[k][:, 384 * half:384 * (half + 1)],
                        start=(k == 0), stop=(k == 5))
                nc.vector.tensor_tensor(
                    out=_ap(vt, vt.offset + 65 * 6 * half,
                            [[vt.ap[0][0], n2c], [65, 6], [1, 64]]),
                    in0=ps[0:n2c, 0:384],
                    in1=vb_rep[0:n2c, 384 * half:384 * (half + 1)],
                    op=mybir.AluOpType.add)

            def emit_proj(b, ci, half, ys):
                """proj for batch b, token chunk ci, 384-col half."""
                n2o, n2c = N1C[ci]
                to = N_TOK * b + n2o
                ps = mmps.tile([128, 512], dt.float32, name="mm", tag="mm")
                for k in range(6):
                    nc.tensor.matmul(ps[0:n2c, 0:384],
                                     ao[k][:, to:to + n2c],
                                     wp[k][:, 384 * half:384 * (half + 1)],
                                     start=(k == 0), stop=(k == 5))
                nc.vector.tensor_tensor(
                    out=ys[0:n2c, 384 * half:384 * (half + 1)],
                    in0=ps[0:n2c, 0:384],
                    in1=pb_rep[0:n2c, 384 * half:384 * (half + 1)],
                    op=mybir.AluOpType.add)

            def proj_units(b):
                ys = [ysp.tile([128, DIM], dt.float32, name="ys0", tag="ys0"),
                      ysp.tile([69, DIM], dt.float32, name="ys1", tag="ys1")]
                units = []
                for ci in range(2):
                    for half in range(2):
                        units.append(lambda b=b, ci=ci, half=half:
                                     emit_proj(b, ci, half, ys[ci]))

                    def out_dma(b=b, ci=ci):
                        n2o, n2c = N1C[ci]
                        nc.sync.dma_start(
                            out=y_d[N_TOK * b + n2o:N_TOK * b + n2o + n2c, :],
                            in_=ys[ci][0:n2c, :])
                    units.append(out_dma)
                return units

            # ---------- attention ----------
            SOFF = [0, 256]

            def emit_scores(b, hp):
                """S^T + bias -> exp for head pair hp of batch b.
                One psum tile per head (hi): a psum bank must only ever be
                written by matmuls of a single tile_position mode --
                mixing (0,0) and (64,0) groups in one bank wedges the PE."""
                sph = [mmps.tile([128, 512], dt.float32, name="mm", tag="mm")
                       for _ in range(2)]
                qt = qk_sb[hp]
                kt = qk_sb[6 + hp]
                for hi in range(2):
                    po = 64 * hi
                    for ci, (n2o, n2c) in enumerate(N1C):
                        nc.tensor.matmul(
                            sph[hi][0:n2c, SOFF[ci]:SOFF[ci] + 197],
                            kt[po:po + 64,
                               N_TOK * b + n2o:N_TOK * b + n2o + n2c],
                            qt[po:po + 64, N_TOK * b:N_TOK * (b + 1)],
                            start=True, stop=True)
                ss0 = ss0p.tile([128, 394], dt.float16, name="ss0", tag="ss0")
                ss1 = ss1p.tile([69, 394], dt.float16, name="ss1", tag="ss1")
                ssx = (ss0, ss1)
                bTx = (bT0_sb, bT1_sb)
                for ci, (n2o, n2c) in enumerate(N1C):
                    for hi in range(2):
                        if "bias" in F:
                            nc.vector.tensor_tensor(
                                out=ssx[ci][0:n2c, 197 * hi:197 * (hi + 1)],
                                in0=sph[hi][0:n2c, SOFF[ci]:SOFF[ci] + 197],
                                in1=bTx[ci][hp][0:n2c, 197 * hi:197 * (hi + 1)],
                                op=mybir.AluOpType.add)
                        else:
                            nc.vector.tensor_copy(
                                ssx[ci][0:n2c, 197 * hi:197 * (hi + 1)],
                                sph[hi][0:n2c, SOFF[ci]:SOFF[ci] + 197])
                et0 = et0p.tile([128, 394], dt.bfloat16, name="et0", tag="et0")
                et1 = et1p.tile([69, 394], dt.bfloat16, name="et1", tag="et1")
                if "exp" in F:
                    nc.scalar.activation(out=et0[:, :], in_=ss0[:, :], func=Exp)
                    nc.scalar.activation(out=et1[:, :], in_=ss1[:, :], func=Exp)
                else:
                    nc.vector.tensor_copy(et0[:, :], ss0[:, :])
                    nc.vector.tensor_copy(et1[:, :], ss1[:, :])
                if debug and b == 0 and hp == 0:
                    nc.sync.dma_start(out=dbg_ss0_d[:, :], in_=ss0[:, :])
                    nc.sync.dma_start(out=dbg_ss1_d[:, :], in_=ss1[:, :])
                    nc.sync.dma_start(out=dbg_et0_d[:, :], in_=et0[:, :])
                    nc.sync.dma_start(out=dbg_et1_d[:, :], in_=et1[:, :])
                return et0, et1

            def emit_av(b, hp, et0, et1, rd_t, anums):
                """AV matmuls; stash numerator (bf16) in sbuf and ship the
                denominator row to the batch's DRAM staging buffer."""
                ap_ = avps.tile([128, 512], dt.float32, name="av", tag="av")
                for hi in range(2):
                    h = 2 * hp + hi
                    for ci, (n2o, n2c) in enumerate(N1C):
                        et = (et0, et1)[ci]
                        nc.tensor.matmul(
                            ap_[0:65, 197 * hi:197 * (hi + 1)],
                            v_sb[b][ci][:, 65 * h:65 * (h + 1)],
                            et[0:n2c, 197 * hi:197 * (hi + 1)],
                            start=(ci == 0), stop=(ci == 1))
                if "rt" not in F:
                    for hi in range(2):
                        nc.vector.tensor_copy(
                            ao[hp][64 * hi:64 * (hi + 1),
                                   N_TOK * b:N_TOK * (b + 1)],
                            ap_[0:64, 197 * hi:197 * (hi + 1)])
                    return
                anum = anump.tile([64, 394], dt.bfloat16, name="an", tag="an")
                nc.vector.tensor_copy(anum[0:64, :], ap_[0:64, 0:394])
                rc = rcp.tile([65, 396], dt.float32, name="rc", tag="rc")
                nc.vector.tensor_copy(rc[64:65, 0:394], ap_[64:65, 0:394])
                nc.vector.tensor_copy(rc[64:65, 394:396], ap_[64:65, 0:2])
                nc.sync.dma_start(out=rd_t[0:1, 396 * hp:396 * (hp + 1)],
                                  in_=rc[64:65, 0:396])
                anums.append((hp, anum))

            def finish_batch(b, rd_t, anums):
                """One wrapped reciprocal for the whole batch's denominators
                (2376 = 99*24 elements), then broadcast + gpsimd multiplies."""
                rw = rwp.tile([99, 24], dt.float32, name="rw", tag="rw")
                nc.sync.dma_start(out=rw[0:99, :],
                                  in_=_ap(rd_t, rd_t.offset, [[24, 99], [1, 24]]))
                rwr = rwp.tile([99, 24], dt.float32, name="rwr", tag="rwr")
                nc.vector.reciprocal(rwr[0:99, :], rw[0:99, :])
                rd2_t = dramp.tile([1, 2376], dt.float32, name="rd2", tag="rd2")
                nc.sync.dma_start(out=_ap(rd2_t, rd2_t.offset,
                                          [[24, 99], [1, 24]]),
                                  in_=rwr[0:99, :])
                for hp, anum in anums:
                    rb = rbp.tile([64, 394], dt.float32, name="rb", tag="rb")
                    nc.sync.dma_start(
                        out=rb[0:64, :],
                        in_=_ap(rd2_t, rd2_t.offset + 396 * hp,
                                [[0, 64], [1, 394]]))
                    if debug and b == 0 and hp == 0:
                        nc.sync.dma_start(out=dbg_rc_d[0:1, :],
                                          in_=_ap(rd2_t, rd2_t.offset,
                                                  [[0, 1], [1, 394]]))
                        nc.sync.dma_start(out=dbg_rb_d[0:64, :],
                                          in_=rb[0:64, :])
                    for hi in range(2):
                        nc.gpsimd.tensor_tensor(
                            out=ao[hp][64 * hi:64 * (hi + 1),
                                       N_TOK * b:N_TOK * (b + 1)],
                            in0=anum[0:64, 197 * hi:197 * (hi + 1)],
                            in1=rb[0:64, 197 * hi:197 * (hi + 1)],
                            op=mybir.AluOpType.mult)

            # ---------- main schedule ----------
            # prologue: qkv for chunk 0 (batches 0,1), dense
            for m in range(12):
                emit_qk_m(0, m)
            for b in (0, 1):
                for ci in range(2):
                    for half in range(2):
                        emit_v(b, ci, half)

            # per chunk: attention for its 2 batches, with next-chunk qkv and
            # previous-batch proj emitted as PE filler between pair stages.
            for c in range(nchunks):
                filler = []
                if c + 1 < nchunks:
                    filler += [lambda m=m, c=c: emit_qk_m(c + 1, m)
                               for m in range(12)]
                    for b in (2 * (c + 1), 2 * (c + 1) + 1):
                        filler += [lambda b=b, ci=ci, half=half:
                                   emit_v(b, ci, half)
                                   for ci in range(2) for half in range(2)]
                if c >= 1 and "proj" in F:
                    filler += proj_units(2 * (c - 1))
                    filler += proj_units(2 * (c - 1) + 1)
                fit = iter(filler)

                def fill(n=1):
                    if "fill" not in F:
                        return
                    for _ in range(n):
                        u = next(fit, None)
                        if u is not None:
                            u()

                for b in (2 * c, 2 * c + 1):
                    if "scores" not in F:
                        continue
                    rd_t = dramp.tile([1, 2376], dt.float32, name="rd",
                                      tag="rd")
                    anums = []
                    pend = []  # (hp, et0, et1) awaiting AV
                    for hp in range(6):
                        ets = emit_scores(b, hp)
                        fill(2)
                        pend.append((hp, ets))
                        if "av" in F and len(pend) >= 2:
                            php, (e0, e1) = pend.pop(0)
                            emit_av(b, php, e0, e1, rd_t, anums)
                            fill(1)
                    if "av" in F:
                        for php, (e0, e1) in pend:
                            emit_av(b, php, e0, e1, rd_t, anums)
                            fill(1)
                        if "rt" in F:
                            finish_batch(b, rd_t, anums)
                # drain leftover filler before next chunk's attention
                for u in fit:
                    u()

            # epilogue: proj for the last two batches
            if "proj" in F:
                for u in proj_units(nb - 2):
                    u()
                for u in proj_units(nb - 1):
                    u()

            if debug:
                for m in range(12):
                    nc.sync.dma_start(out=dbg_qk_d[128 * m:128 * (m + 1), :],
                                      in_=qk_sb[m][:, :])
                for ci in range(2):
                    n2c = N1C[ci][1]
                    nc.sync.dma_start(
                        out=dbg_v_d[128 * ci:128 * ci + n2c, :],
                        in_=v_sb[0][ci][:, :])
                for m in range(6):
                    nc.sync.dma_start(out=dbg_ao_d[128 * m:128 * (m + 1), :],
                                      in_=ao[m][:, :])

    nc.compile()
    return nc


def _marshal(x, qkv_w, q_bias, v_bias, rpb_table, proj_w, proj_b, rel_index):
    B = x.shape[0]
    ncore = 8
    bpc = B // ncore

    wqkT = np.ascontiguousarray(qkv_w[0:2 * DIM, :].T.astype(np.float32))
    wqkT[:, 0:DIM] *= SCALE
    wqkT = wqkT.astype(BF16)
    wvT = np.ascontiguousarray(qkv_w[2 * DIM:3 * DIM, :].T.astype(BF16))
    wpT = np.ascontiguousarray(proj_w.T.astype(BF16))
    qb2 = np.ascontiguousarray(
        (q_bias.astype(np.float32) * SCALE).reshape(6, 128).T)

    # full transposed bias, pair-packed: bias[h][n1, n2] -> biasT[h][n2, n1]
    bias = rpb_table[np.asarray(rel_index).reshape(-1)].reshape(
        N_TOK, N_TOK, NH).astype(np.float32)  # [n1, n2, h]
    bT0 = np.zeros((6 * 128, 394), dtype=BF16)
    bT1 = np.zeros((6 * 69, 394), dtype=BF16)
    for hp in range(6):
        for hi in range(2):
            bt = bias[:, :, 2 * hp + hi].T  # [n2, n1]
            bT0[128 * hp:128 * (hp + 1), 197 * hi:197 * (hi + 1)] = bt[0:128, :]
            bT1[69 * hp:69 * (hp + 1), 197 * hi:197 * (hi + 1)] = bt[128:197, :]

    shared = {"wqkT": wqkT, "wvT": wvT, "wpT": wpT, "qb2": qb2,
              "vb": np.ascontiguousarray(v_bias.astype(np.float32)),
              "pb": np.ascontiguousarray(proj_b.astype(np.float32)),
              "bT0": bT0, "bT1": bT1}
    x2 = np.asarray(x, dtype=np.float32).reshape(B, N_TOK, DIM)
    in_maps = []
    for c in range(ncore):
        xTb = np.ascontiguousarray(
            x2[c * bpc:(c + 1) * bpc].reshape(bpc * N_TOK, DIM).T.astype(BF16))
        m = dict(shared)
        m["xTb"] = xTb
        in_maps.append(m)
    return in_maps, bpc


last_exec_time_ns = None
last_results = None


def _install_ntff_hook():
    """Provide antenv.axon_hooks + register the ctypes NTFF hook (the agent
    image's antenv lacks axon_hooks, so trn_boot degraded silently)."""
    import types
    import contextlib
    import ctypes

    try:
        from antenv.axon_hooks import get_axon_ntff_profile_hook
        if get_axon_ntff_profile_hook() is not None:
            return
    except ImportError:
        import antenv
        mod = types.ModuleType("antenv.axon_hooks")
        mod._hook = None

        def set_axon_ntff_profile_hook(h):
            mod._hook = h

        def get_axon_ntff_profile_hook():
            return mod._hook

        mod.set_axon_ntff_profile_hook = set_axon_ntff_profile_hook
        mod.get_axon_ntff_profile_hook = get_axon_ntff_profile_hook
        sys.modules["antenv.axon_hooks"] = mod
        antenv.axon_hooks = mod

    so_path = "/opt/axon/libaxon_pjrt.so"
    lib = ctypes.CDLL(so_path)
    if not hasattr(lib, "axon_start_nrt_profile"):
        return
    lib.axon_start_nrt_profile.argtypes = [ctypes.POINTER(ctypes.c_int64),
                                           ctypes.c_size_t]
    lib.axon_start_nrt_profile.restype = ctypes.c_int64
    lib.axon_stop_nrt_profile.argtypes = [ctypes.c_char_p]
    lib.axon_stop_nrt_profile.restype = ctypes.c_int64

    @contextlib.contextmanager
    def _hook(output_dir, device_ids):
        import jax
        jax.devices()
        if device_ids:
            ids = (ctypes.c_int64 * len(device_ids))(*device_ids)
            rc = lib.axon_start_nrt_profile(ids, len(device_ids))
        else:
            rc = lib.axon_start_nrt_profile(None, 0)
        if rc != 0:
            raise RuntimeError(f"axon_start_nrt_profile rc={rc}")
        try:
            yield
        finally:
            n = lib.axon_stop_nrt_profile(str(output_dir).encode())
            print(f"ntff profile: {n} file(s) -> {output_dir}", file=sys.stderr)

    from antenv.axon_hooks import set_axon_ntff_profile_hook
    set_axon_ntff_profile_hook(_hook)


def kernel(x, qkv_w, q_bias, v_bias, rpb_table, proj_w, proj_b, rel_index):
    global last_exec_time_ns, last_results
    import os
    if os.environ.get("KERNEL_TRACE"):
        _install_ntff_hook()
    from concourse.bass_utils import run_bass_kernel_spmd

    x = np.asarray(x, dtype=np.float32)
    qkv_w = np.asarray(qkv_w, dtype=np.float32)
    q_bias = np.asarray(q_bias, dtype=np.float32)
    v_bias = np.asarray(v_bias, dtype=np.float32)
    rpb_table = np.asarray(rpb_table, dtype=np.float32)
    proj_w = np.asarray(proj_w, dtype=np.float32)
    proj_b = np.asarray(proj_b, dtype=np.float32)

    B = x.shape[0]
    bpc = B // 8
    if 'nc' not in _cache:
        _cache['nc'] = build_program(bpc)
    nc = _cache['nc']

    in_maps, bpc = _marshal(x, qkv_w, q_bias, v_bias, rpb_table,
                            proj_w, proj_b, rel_index)
    res = run_bass_kernel_spmd(nc, in_maps, core_ids=list(range(8)),
                               trace=bool(os.environ.get("KERNEL_TRACE")))
    last_exec_time_ns = res.exec_time_ns
    last_results = res
    ys = [res.results[c]["y"].reshape(bpc, N_TOK, DIM) for c in range(8)]
    return np.concatenate(ys, axis=0).astype(np.float32)
